# revision 41
# baseline (speedup 1.0000x reference)
"""Bass/Tile kernel for nn_AttnModule (sparse_attention).

Reference computation (per batch b):
    scores  = pos_emb @ position_fmap[b].T          # [T, L]
    attn    = softmax(scores, axis=-1)              # softmax over L
    context = attn @ origin_fmap[b]                 # [T, H]
    out     = context @ W_gen.T + b_gen             # [T, C]

Sharding: pure data parallel over batch B=64 -> 8 cores x 8 batches.

Measured cost model (this hw, via differential For_i timing):
  - every PE matmul instruction costs ~59ns on top of its N-cycle
    stream @2.4GHz, regardless of grouping/stationary reuse; a
    transpose costs ~168ns standalone but only ~94ns marginal when
    interleaved between N=512 matmul streams.
  - HBM DMA sustains ~340GB/s per core only with big (>=0.5MB)
    dma_start entries on one queue; 4-way splits drop to ~230GB/s
    and multi-queue spreading is slower.
  - the PE HAM clock throttle (1.2GHz cold / 2.4GHz warm, 3.4us
    free-running windows) makes idle-interleaved schedules ~2x slow.

v2 design vs the hi/lo baseline (65.3us -> 47.2us same-method):
  - pos_emb single fp16 term (mm1 8192 -> 4096 cyc/batch); logit
    rounding error from fp16 pf dominates anyway and stays ~1.4e-2.
  - origin_fmap streamed as fp8 E3M4 (4 mantissa bits, data ~N(0,1)):
    halves the of DMA bytes; mm2 takes the fp8 moving operand at fp16
    rate (mixed-dtype matmul), PSUM accumulates fp32.
  - TP=112 (T=100 padded) instead of 128 trims transpose/mm3 streams.
  - mm3 batched over groups of 4 batches (one [C,4*TP] PSUM tile) to
    amortize the wgT weight load.
  - software-pipelined schedule: step i emits [mm1(i) with attn-T(i-1)
    transposes interleaved] then [mm2(i-PIPE) with ctx-T(i-PIPE-1)
    interleaved + mm3]; the DMA queue is ordered pf0..pf2, (pf3,of0),
    (pf4,of1).. so arrivals are just-in-time for both phases.
  - per-batch PE floor reached: 16 stream MMs + 12 interleaved
    transposes + mm3/4 = ~5.3us/batch incl. instruction overheads.

Layout choices (host side prep is free):
  - position_fmap shipped pre-transposed per batch: pfT [B, 128, HT, L]
  - pos_emb shipped transposed fp16, T zero-padded to TP: peT [1, H, TP]
  - origin_fmap shipped l-tiled fp8e3: of [B, 128, LT, H]
  - W_gen shipped transposed: wgT [H, C]
  - output produced as [B, C, TP] fp16, transposed back on host.
"""

import numpy as np
import ml_dtypes

import concourse.mybir as mybir
import concourse.tile as tile
from concourse import bacc
from concourse.bass_utils import run_bass_kernel_spmd
from concourse.masks import make_identity

B, L, H, T, C = 64, 1024, 512, 100, 97
TP = 112
NCORES = 8
BPC = B // NCORES  # batches per core

HT = H // 128  # 4 h-tiles
LT = L // 128  # 8 l-tiles

F32 = mybir.dt.float32
AF = mybir.ActivationFunctionType
AX = mybir.AxisListType
OP = mybir.AluOpType

MM_DT = mybir.dt.float16
NP_DT = np.float16
OF_DT = mybir.dt.float8e3
OF_NP = ml_dtypes.float8_e3m4
PE_TERMS = 1
PIPE = 3  # back(b-PIPE) interleaves with front(b)
MM3_GRP = 4


def build_nc(mm_dt=MM_DT, of_dt=OF_DT, pe_terms=PE_TERMS, repeats=1, pipe=PIPE, hw_loop=None, only=None, dma_mode="single", unroll=8, order_swap=False):
    """hw_loop=N wraps the body in a For_i hardware loop (timing builds):
    outT becomes an Internal scratch and a tiny token is the only external
    output, so per-call wire traffic is negligible and device time
    dominates."""
    timing = hw_loop is not None
    nc = bacc.Bacc(None, target_bir_lowering=False, debug=False)

    pfT = nc.dram_tensor("pfT", [BPC, 128, HT, L], mm_dt, kind="ExternalInput").ap()
    of = nc.dram_tensor("of", [BPC, 128, LT, H], of_dt, kind="ExternalInput").ap()
    peT = nc.dram_tensor("peT", [pe_terms, H, TP], mm_dt, kind="ExternalInput").ap()
    wgT = nc.dram_tensor("wgT", [H, C], mm_dt, kind="ExternalInput").ap()
    bg = nc.dram_tensor("bg", [C, 1], F32, kind="ExternalInput").ap()
    if timing:
        outT = nc.dram_tensor("outT", [BPC, C, TP], mm_dt, kind="Internal").ap()
        tok = nc.dram_tensor("tok", [C, 1], F32, kind="ExternalOutput").ap()
    else:
        outT = nc.dram_tensor("outT", [BPC, C, TP], mm_dt, kind="ExternalOutput").ap()

    with tile.TileContext(nc) as tc:
        with (
            tc.tile_pool(name="consts", bufs=1) as consts,
            tc.tile_pool(name="pf", bufs=BPC) as pfpool,
            tc.tile_pool(name="ofp", bufs=BPC) as ofpool,
            tc.tile_pool(name="mid", bufs=pipe + 2) as mid,
            tc.tile_pool(name="work", bufs=3) as work,
            tc.tile_pool(name="cts", bufs=2) as ctspool,
            tc.tile_pool(name="ps_scores", bufs=3, space="PSUM") as ps_scores,
            tc.tile_pool(name="ps_tp", bufs=2, space="PSUM") as ps_tp,
            tc.tile_pool(name="ps_ctx", bufs=2, space="PSUM") as ps_ctx,
            tc.tile_pool(name="ps_out", bufs=1, space="PSUM") as ps_out,
        ):
            # ---- constants ----
            peT_sb = consts.tile([128, pe_terms, HT, TP], mm_dt)
            peTr = peT.rearrange("e (ht p) t -> p e ht t", p=128)
            for e in range(pe_terms):
                nc.sync.dma_start(peT_sb[:, e], peTr[:, e])
            wgT_sb = consts.tile([128, HT, C], mm_dt)
            bg_sb = consts.tile([C, 1], F32)
            ident = consts.tile([128, 128], mm_dt)
            make_identity(nc, ident)

            def load_pf(b):
                pf_sb = pfpool.tile([128, HT, L], mm_dt, tag="pf")
                if dma_mode == "split4":
                    # per-ht DMAs: per-partition runs are contiguous 2KB
                    for ht in range(HT):
                        nc.sync.dma_start(pf_sb[:, ht, :], pfT[b, :, ht, :])
                elif dma_mode in ("single", "2q_big"):
                    # one 1MB dma_start; per-partition run = 8KB contiguous
                    nc.sync.dma_start(pf_sb, pfT[b])
                elif dma_mode == "act_single":
                    nc.scalar.dma_start(pf_sb, pfT[b])
                elif dma_mode == "halves":
                    nc.sync.dma_start(pf_sb[:, :2, :], pfT[b, :, :2, :])
                    nc.sync.dma_start(pf_sb[:, 2:, :], pfT[b, :, 2:, :])
                elif dma_mode == "2q":
                    nc.sync.dma_start(pf_sb[:, :2, :], pfT[b, :, :2, :])
                    nc.scalar.dma_start(pf_sb[:, 2:, :], pfT[b, :, 2:, :])
                elif dma_mode == "3q":
                    nc.sync.dma_start(pf_sb[:, :2, :], pfT[b, :, :2, :])
                    nc.scalar.dma_start(pf_sb[:, 2, :], pfT[b, :, 2, :])
                    nc.gpsimd.dma_start(pf_sb[:, 3, :], pfT[b, :, 3, :])
                return pf_sb

            def load_of(b):
                of_sb = ofpool.tile([128, LT, H], of_dt, tag="of")
                if dma_mode == "split4":
                    for i in range(4):
                        nc.sync.dma_start(
                            of_sb[:, 2 * i : 2 * (i + 1), :], of[b, :, 2 * i : 2 * (i + 1), :]
                        )
                elif dma_mode == "single":
                    nc.sync.dma_start(of_sb, of[b])
                elif dma_mode == "2q_big":
                    nc.scalar.dma_start(of_sb, of[b])
                elif dma_mode == "act_single":
                    nc.scalar.dma_start(of_sb, of[b])
                elif dma_mode == "halves":
                    nc.sync.dma_start(of_sb, of[b])
                elif dma_mode == "2q":
                    nc.sync.dma_start(of_sb[:, :4, :], of[b, :, :4, :])
                    nc.scalar.dma_start(of_sb[:, 4:, :], of[b, :, 4:, :])
                elif dma_mode == "3q":
                    nc.scalar.dma_start(of_sb[:, :4, :], of[b, :, :4, :])
                    nc.gpsimd.dma_start(of_sb[:, 4:, :], of[b, :, 4:, :])
                return of_sb

            def front(i, pf_sb, prevT):
                """mm1(i) with attn-T(i-1) transposes interleaved between the
                matmul streams (each transpose's LDWEIGHTS hides under the
                neighboring N=512 stream: ~366ns/pair vs 440 separate), then
                the softmax chain for i on DVE/ACT. Returns (p, rinv) of i;
                stores pT(i-1) into state."""
                tp_ps = None
                if prevT is not None:
                    p_prev, rinv_prev = prevT
                    tp_ps = ps_tp.tile([128, LT, TP], mm_dt, tag="tp", name="tp")
                    pT_sb = mid.tile([128, LT, TP], mm_dt, tag="pT", name="pT")
                k = 0
                if pf_sb is not None:
                    sc_ps = [
                        ps_scores.tile([TP, 512], F32, tag="scores", name=f"sc{lh}")
                        for lh in range(L // 512)
                    ]
                    for lh in range(L // 512):
                        for ht in range(HT):
                            nc.tensor.matmul(
                                sc_ps[lh],
                                lhsT=peT_sb[:, 0, ht, :],
                                rhs=pf_sb[:, ht, lh * 512 : (lh + 1) * 512],
                                start=(ht == 0),
                                stop=(ht == HT - 1),
                            )
                            if tp_ps is not None and k < LT:
                                nc.tensor.transpose(
                                    tp_ps[:, k, :],
                                    p_prev[:, k * 128 : (k + 1) * 128],
                                    ident[:TP, :TP],
                                )
                                k += 1
                if tp_ps is not None:
                    while k < LT:
                        nc.tensor.transpose(
                            tp_ps[:, k, :], p_prev[:, k * 128 : (k + 1) * 128], ident[:TP, :TP]
                        )
                        k += 1
                    half = LT // 2
                    nc.vector.tensor_copy(pT_sb[:, :half, :], tp_ps[:, :half, :])
                    nc.scalar.copy(pT_sb[:, half:, :], tp_ps[:, half:, :])
                    state[i - 1] = (pT_sb, rinv_prev)

                if pf_sb is None:
                    return None
                m2 = work.tile([TP, 2], F32, tag="m2")
                for lh in range(L // 512):
                    nc.vector.tensor_reduce(m2[:, lh : lh + 1], sc_ps[lh], axis=AX.X, op=OP.max)
                negm = work.tile([TP, 1], F32, tag="negm")
                nc.vector.tensor_reduce(negm, m2, axis=AX.X, op=OP.max, negate=True)
                p_sb = work.tile([TP, L], mm_dt, tag="p")
                s2 = work.tile([TP, 2], F32, tag="s2")
                for lh in range(L // 512):
                    nc.scalar.activation(
                        p_sb[:, lh * 512 : (lh + 1) * 512],
                        sc_ps[lh],
                        AF.Exp,
                        bias=negm,
                        scale=1.0,
                        accum_out=s2[:, lh : lh + 1],
                    )
                ssum = work.tile([TP, 1], F32, tag="ssum")
                nc.vector.tensor_reduce(ssum, s2, axis=AX.X, op=OP.add)
                rinv = mid.tile([TP, 1], F32, tag="rinv")
                nc.vector.reciprocal(rinv, ssum)
                return p_sb, rinv

            def back_mm2(bb, of_sb, pT_sb, rinv):
                """mm2(bb) with ctx-T(bb-1) transposes interleaved; then the
                rinv-scaled PSUM copy-out for bb."""
                prev_ctx = ctxs.pop(bb - 1, None)
                tp_ps = None
                if prev_ctx is not None:
                    tp_ps = ps_tp.tile([128, LT, TP], mm_dt, tag="tp", name="tpc")
                k = 0
                ctx_ps = ps_ctx.tile([TP, H], F32, tag="ctx")
                for lt in range(LT):
                    nc.tensor.matmul(
                        ctx_ps,
                        lhsT=pT_sb[:, lt, :],
                        rhs=of_sb[:, lt, :],
                        start=(lt == 0),
                        stop=(lt == LT - 1),
                    )
                    if tp_ps is not None and k < HT:
                        nc.tensor.transpose(
                            tp_ps[:, k, :],
                            prev_ctx[:, k * 128 : (k + 1) * 128],
                            ident[:TP, :TP],
                        )
                        k += 1
                if tp_ps is not None:
                    g = (bb - 1) // MM3_GRP
                    nc.scalar.copy(cT4[g][:, :, (bb - 1) % MM3_GRP, :], tp_ps[:, :HT, :])
                    if (bb - 1) % MM3_GRP == MM3_GRP - 1:
                        back_mm3(g * MM3_GRP, MM3_GRP, cT4[g])
                ctx_sb = work.tile([TP, H], mm_dt, tag="ctx_sb")
                nc.vector.tensor_scalar_mul(ctx_sb, ctx_ps[:], rinv)
                return ctx_sb

            def final_tp(bb):
                """drain: ctx transpose + copy + mm3 for the last group."""
                prev_ctx = ctxs.pop(bb)
                tp_ps = ps_tp.tile([128, LT, TP], mm_dt, tag="tp", name="tpc")
                for k in range(HT):
                    nc.tensor.transpose(
                        tp_ps[:, k, :], prev_ctx[:, k * 128 : (k + 1) * 128], ident[:TP, :TP]
                    )
                g = bb // MM3_GRP
                nc.scalar.copy(cT4[g][:, :, bb % MM3_GRP, :], tp_ps[:, :HT, :])
                back_mm3(g * MM3_GRP, BPC - g * MM3_GRP, cT4[g])

            def back_mm3(b0, nb, cT4_sb):
                """mm3 + bias + store for batches b0..b0+nb-1."""
                o_ps = ps_out.tile([C, MM3_GRP * TP], F32, tag="o")
                for ht in range(HT):
                    nc.tensor.matmul(
                        o_ps[:, : nb * TP],
                        lhsT=wgT_sb[:, ht, :],
                        rhs=cT4_sb[:, ht, :nb, :],
                        start=(ht == 0),
                        stop=(ht == HT - 1),
                    )
                out_sb = work.tile([C, MM3_GRP, TP], mm_dt, tag="out_sb")
                nc.vector.tensor_scalar_add(
                    out_sb[:, :nb, :],
                    o_ps[:, : nb * TP].rearrange("c (b t) -> c b t", b=nb),
                    bg_sb,
                )
                nc.gpsimd.dma_start(
                    outT[b0 : b0 + nb].rearrange("b c t -> c b t"), out_sb[:, :nb, :]
                )

            state = {}
            ctxs = {}
            cT4 = {}

            nodma_tiles = None
            if only == "nodma":
                pf0_sb = consts.tile([128, HT, L], mm_dt, name="pf0c")
                of0_sb = consts.tile([128, LT, H], of_dt, name="of0c")
                nc.sync.dma_start(pf0_sb, pfT[0])
                nc.sync.dma_start(of0_sb, of[0])
                nodma_tiles = (pf0_sb, of0_sb)

            def body(load_consts):
                if only == "empty":
                    nc.vector.tensor_copy(bg_sb, bg_sb)
                    return
                if only == "pe":
                    # dense independent matmuls: 64 x 512 rows = 32768 PE
                    # cycles -> 13.6us warm / 27.3us cold
                    pf_sb = pfpool.tile([128, HT, L], mm_dt, tag="pf", name="pf")
                    nc.sync.dma_start(pf_sb, pfT[0])
                    for i in range(64):
                        sc = ps_scores.tile([TP, 512], F32, tag="scores", name="sc")
                        nc.tensor.matmul(
                            sc,
                            lhsT=peT_sb[:, 0, i % HT, :],
                            rhs=pf_sb[:, i % HT, (i % 2) * 512 : (i % 2 + 1) * 512],
                            start=True,
                            stop=True,
                        )
                    return
                if only == "pe_grp":
                    # 16 groups of 4 accumulating MMs (64 MMs, N=512)
                    pf_sb = pfpool.tile([128, HT, L], mm_dt, tag="pf", name="pf")
                    nc.sync.dma_start(pf_sb, pfT[0])
                    for g in range(16):
                        sc = ps_scores.tile([TP, 512], F32, tag="scores", name="sc")
                        for j in range(4):
                            nc.tensor.matmul(
                                sc,
                                lhsT=peT_sb[:, 0, j, :],
                                rhs=pf_sb[:, j, (g % 2) * 512 : (g % 2 + 1) * 512],
                                start=(j == 0),
                                stop=(j == 3),
                            )
                    return
                if only == "pe_tp":
                    # 64 transposes of [TP,128] -> overhead probe
                    p_sb = work.tile([TP, L], mm_dt, tag="p", name="p")
                    nc.sync.dma_start(p_sb, pfT[0, :TP, 0, :])
                    for i in range(64):
                        tp_ps = ps_tp.tile([128, TP], mm_dt, tag="tp", name="tp")
                        nc.tensor.transpose(
                            tp_ps, p_sb[:, (i % 8) * 128 : (i % 8 + 1) * 128], ident[:TP, :TP]
                        )
                    return
                if only == "pe_n1024":
                    # 32 MMs with N=1024 out (2 PSUM banks) - legality probe
                    pf_sb = pfpool.tile([128, HT, L], mm_dt, tag="pf", name="pf")
                    nc.sync.dma_start(pf_sb, pfT[0])
                    for i in range(32):
                        sc = ps_scores.tile([TP, 1024], F32, tag="sc1k", name="sc")
                        nc.tensor.matmul(
                            sc,
                            lhsT=peT_sb[:, 0, i % HT, :],
                            rhs=pf_sb[:, i % HT, :],
                            start=True,
                            stop=True,
                        )
                    return
                if only == "pe_acc":
                    # 64 MMs N=512 accumulating into ONE psum tile: no
                    # per-MM WAR semaphores, one group
                    pf_sb = pfpool.tile([128, HT, L], mm_dt, tag="pf", name="pf")
                    nc.sync.dma_start(pf_sb, pfT[0])
                    sc = ps_scores.tile([TP, 512], F32, tag="scores", name="sc")
                    for i in range(64):
                        nc.tensor.matmul(
                            sc,
                            lhsT=peT_sb[:, 0, i % HT, :],
                            rhs=pf_sb[:, i % HT, (i % 2) * 512 : (i % 2 + 1) * 512],
                            start=(i == 0),
                            stop=(i == 63),
                        )
                    nc.vector.tensor_reduce(
                        work.tile([TP, 1], F32, tag="m2", name="m2"), sc, axis=AX.X, op=OP.max
                    )
                    return
                if only == "pe_same":
                    # 64 MMs N=512, all with the SAME stationary operand
                    pf_sb = pfpool.tile([128, HT, L], mm_dt, tag="pf", name="pf")
                    nc.sync.dma_start(pf_sb, pfT[0])
                    for i in range(64):
                        sc = ps_scores.tile([TP, 512], F32, tag="scores", name="sc")
                        nc.tensor.matmul(
                            sc,
                            lhsT=peT_sb[:, 0, 0, :],
                            rhs=pf_sb[:, i % HT, (i % 2) * 512 : (i % 2 + 1) * 512],
                            start=True,
                            stop=True,
                        )
                    return
                if only == "pe_mix":
                    # 32 MMs N=512 alternating with 32 transposes
                    pf_sb = pfpool.tile([128, HT, L], mm_dt, tag="pf", name="pf")
                    nc.sync.dma_start(pf_sb, pfT[0])
                    p_sb = work.tile([TP, L], mm_dt, tag="p", name="p")
                    nc.sync.dma_start(p_sb, pfT[0, :TP, 0, :])
                    for i in range(32):
                        sc = ps_scores.tile([TP, 512], F32, tag="scores", name="sc")
                        nc.tensor.matmul(
                            sc,
                            lhsT=peT_sb[:, 0, i % HT, :],
                            rhs=pf_sb[:, i % HT, (i % 2) * 512 : (i % 2 + 1) * 512],
                            start=True,
                            stop=True,
                        )
                        tp_ps = ps_tp.tile([128, TP], mm_dt, tag="tp", name="tp")
                        nc.tensor.transpose(
                            tp_ps, p_sb[:, (i % 8) * 128 : (i % 8 + 1) * 128], ident[:TP, :TP]
                        )
                    return
                # ---- DMA queue order: JIT arrivals ----
                pf_tiles = {}
                of_tiles = {}
                if only == "nodma":
                    for b in range(BPC):
                        pf_tiles[b] = nodma_tiles[0]
                        of_tiles[b] = nodma_tiles[1]
                else:
                    for b in range(min(pipe, BPC)):
                        pf_tiles[b] = load_pf(b)
                    if load_consts:
                        nc.sync.dma_start(wgT_sb, wgT.rearrange("(ht p) c -> p ht c", p=128))
                        nc.sync.dma_start(bg_sb, bg)
                    for b in range(pipe, BPC):
                        pf_tiles[b] = load_pf(b)
                        of_tiles[b - pipe] = load_of(b - pipe)
                    for b in range(max(BPC - pipe, 0), BPC):
                        of_tiles[b] = load_of(b)
                if only == "dma":
                    return

                # ---- compute: PE order per step i is
                #   [mm1(i) ⊗ attn-T(i-1)] | [mm2(i-pipe) ⊗ ctx-T(i-pipe-1), mm3]
                # (⊗ = transposes interleaved between matmul streams so their
                # LDWEIGHTS hide; the softmax chain of batch i hides under
                # the back-half matmuls). ----
                state.clear()
                ctxs.clear()
                cT4.clear()
                prev = None
                for i in range(BPC + pipe):
                    if i < BPC:
                        prev = front(i, pf_tiles.pop(i), prev)
                    elif i == BPC:
                        front(i, None, prev)
                        prev = None
                    if i >= pipe:
                        bb = i - pipe
                        if bb % MM3_GRP == 0:
                            cT4[bb // MM3_GRP] = ctspool.tile(
                                [128, HT, MM3_GRP, TP], mm_dt, tag="cT4", name="cT4"
                            )
                        ctxs[bb] = back_mm2(bb, of_tiles.pop(bb), *state.pop(bb))
                final_tp(BPC - 1)

            if timing:
                # unroll several reps per For_i iteration: the loop's
                # all-engine barrier idles PE long enough to re-engage the
                # HAM clock throttle (1.2 GHz), so amortize it to ~1% and
                # measure warm steady-state throughput like an unrolled NEFF
                assert hw_loop % unroll == 0
                nc.sync.dma_start(wgT_sb, wgT.rearrange("(ht p) c -> p ht c", p=128))
                nc.sync.dma_start(bg_sb, bg)
                with tc.For_i(0, hw_loop // unroll, 1):
                    for _u in range(unroll):
                        body(load_consts=False)
                nc.gpsimd.dma_start(tok, bg_sb)
            else:
                for _rep in range(repeats):
                    body(load_consts=(_rep == 0))

    nc.compile()
    return nc


_NC = None


def _get_nc():
    global _NC
    if _NC is None:
        _NC = build_nc()
    return _NC


def make_in_maps(position_fmap, origin_fmap, pos_emb, W_gen, b_gen, np_dt=NP_DT, pe_terms=PE_TERMS, of_np=OF_NP):
    """Host-side sharding + layout prep. Returns list of per-core input dicts."""
    pf = np.asarray(position_fmap, dtype=np.float32)
    of = np.asarray(origin_fmap, dtype=np.float32)
    pe = np.asarray(pos_emb, dtype=np.float32)
    wg = np.asarray(W_gen, dtype=np.float32)
    bgv = np.asarray(b_gen, dtype=np.float32)

    # [B, L, H] -> [B, H, L] -> [B, 128, HT, L]  (partition-major, h = ht*128 + p)
    pfT = np.ascontiguousarray(
        pf.transpose(0, 2, 1).reshape(B, HT, 128, L).transpose(0, 2, 1, 3)
    ).astype(np_dt)
    # [B, L, H] -> [B, 128, LT, H]  (partition-major tiling, l = lt*128 + p)
    of_c = np.ascontiguousarray(
        of.reshape(B, LT, 128, H).transpose(0, 2, 1, 3)
    ).astype(of_np)

    peT_f32 = np.zeros((H, TP), dtype=np.float32)
    peT_f32[:, :T] = pe.T
    terms = []
    resid = peT_f32
    for _ in range(pe_terms):
        t = resid.astype(np_dt)
        terms.append(t)
        resid = resid - t.astype(np.float32)
    peT = np.ascontiguousarray(np.stack(terms, axis=0))  # [pe_terms, H, TP]

    wgT = np.ascontiguousarray(wg.T).astype(np_dt)
    bg2 = np.ascontiguousarray(bgv.reshape(C, 1)).astype(np.float32)

    in_maps = []
    for i in range(NCORES):
        sl = slice(i * BPC, (i + 1) * BPC)
        in_maps.append(
            {
                "pfT": pfT[sl],
                "of": of_c[sl],
                "peT": peT,
                "wgT": wgT,
                "bg": bg2,
            }
        )
    return in_maps


def kernel(position_fmap, origin_fmap, pos_emb, W_gen, b_gen):
    nc = _get_nc()
    in_maps = make_in_maps(position_fmap, origin_fmap, pos_emb, W_gen, b_gen)
    res = run_bass_kernel_spmd(nc, in_maps, core_ids=list(range(NCORES)))
    outs = [r["outT"] for r in res.results]  # each [BPC, C, TP]
    out = np.concatenate(outs, axis=0)[:, :, :T]  # [B, C, T]
    return np.ascontiguousarray(out.transpose(0, 2, 1)).astype(np.float32)


# revision 42
# speedup vs baseline: 1.0613x; 1.0613x over previous
"""Bass/Tile kernel for nn_AttnModule (sparse_attention).

Reference computation (per batch b):
    scores  = pos_emb @ position_fmap[b].T          # [T, L]
    attn    = softmax(scores, axis=-1)              # softmax over L
    context = attn @ origin_fmap[b]                 # [T, H]
    out     = context @ W_gen.T + b_gen             # [T, C]

Sharding: pure data parallel over batch B=64 -> 8 cores x 8 batches.

Measured cost model (this hw, via differential For_i timing):
  - every PE matmul instruction costs ~59ns on top of its N-cycle
    stream @2.4GHz, regardless of grouping/stationary reuse; a
    transpose costs ~168ns standalone but only ~94ns marginal when
    interleaved between N=512 matmul streams.
  - HBM DMA sustains ~340GB/s per core only with big (>=0.5MB)
    dma_start entries on one queue; 4-way splits drop to ~230GB/s
    and multi-queue spreading is slower.
  - the PE HAM clock throttle (1.2GHz cold / 2.4GHz warm, 3.4us
    free-running windows) makes idle-interleaved schedules ~2x slow.

v2 design vs the hi/lo baseline (65.3us -> 47.2us same-method):
  - pos_emb single fp16 term (mm1 8192 -> 4096 cyc/batch); logit
    rounding error from fp16 pf dominates anyway and stays ~1.4e-2.
  - origin_fmap streamed as fp8 E3M4 (4 mantissa bits, data ~N(0,1)):
    halves the of DMA bytes; mm2 takes the fp8 moving operand at fp16
    rate (mixed-dtype matmul), PSUM accumulates fp32.
  - TP=112 (T=100 padded) instead of 128 trims transpose/mm3 streams.
  - mm3 batched over groups of 4 batches (one [C,4*TP] PSUM tile) to
    amortize the wgT weight load.
  - software-pipelined schedule: step i emits [mm1(i) with attn-T(i-1)
    transposes interleaved] then [mm2(i-PIPE) with ctx-T(i-PIPE-1)
    interleaved + mm3]; the DMA queue is ordered pf0..pf2, (pf3,of0),
    (pf4,of1).. so arrivals are just-in-time for both phases.
  - per-batch PE floor reached: 16 stream MMs + 12 interleaved
    transposes + mm3/4 = ~5.3us/batch incl. instruction overheads.

Layout choices (host side prep is free):
  - position_fmap shipped pre-transposed per batch: pfT [B, 128, HT, L]
  - pos_emb shipped transposed fp16, T zero-padded to TP: peT [1, H, TP]
  - origin_fmap shipped l-tiled fp8e3: of [B, 128, LT, H]
  - W_gen shipped transposed: wgT [H, C]
  - output produced as [B, C, TP] fp16, transposed back on host.
"""

import numpy as np
import ml_dtypes

import concourse.mybir as mybir
import concourse.tile as tile
from concourse import bacc
from concourse.bass_utils import run_bass_kernel_spmd
from concourse.masks import make_identity

B, L, H, T, C = 64, 1024, 512, 100, 97
TP = 112
NCORES = 8
BPC = B // NCORES  # batches per core

HT = H // 128  # 4 h-tiles
LT = L // 128  # 8 l-tiles

F32 = mybir.dt.float32
AF = mybir.ActivationFunctionType
AX = mybir.AxisListType
OP = mybir.AluOpType

MM_DT = mybir.dt.float16
NP_DT = np.float16
OF_DT = mybir.dt.float8e3
OF_NP = ml_dtypes.float8_e3m4
PE_TERMS = 1
PIPE = 3  # back(b-PIPE) interleaves with front(b)
MM3_GRP = 4


def build_nc(mm_dt=MM_DT, of_dt=OF_DT, pe_terms=PE_TERMS, repeats=1, pipe=PIPE, hw_loop=None, only=None, dma_mode="single", unroll=8, order_swap=False, deep=False):
    """hw_loop=N wraps the body in a For_i hardware loop (timing builds):
    outT becomes an Internal scratch and a tiny token is the only external
    output, so per-call wire traffic is negligible and device time
    dominates."""
    timing = hw_loop is not None
    nc = bacc.Bacc(None, target_bir_lowering=False, debug=False)

    pfT = nc.dram_tensor("pfT", [BPC, 128, HT, L], mm_dt, kind="ExternalInput").ap()
    of = nc.dram_tensor("of", [BPC, 128, LT, H], of_dt, kind="ExternalInput").ap()
    peT = nc.dram_tensor("peT", [pe_terms, H, TP], mm_dt, kind="ExternalInput").ap()
    wgT = nc.dram_tensor("wgT", [H, C], mm_dt, kind="ExternalInput").ap()
    bg = nc.dram_tensor("bg", [C, 1], F32, kind="ExternalInput").ap()
    if timing:
        outT = nc.dram_tensor("outT", [BPC, C, TP], mm_dt, kind="Internal").ap()
        tok = nc.dram_tensor("tok", [C, 1], F32, kind="ExternalOutput").ap()
    else:
        outT = nc.dram_tensor("outT", [BPC, C, TP], mm_dt, kind="ExternalOutput").ap()

    with tile.TileContext(nc) as tc:
        with (
            tc.tile_pool(name="consts", bufs=1) as consts,
            tc.tile_pool(name="pf", bufs=BPC) as pfpool,
            tc.tile_pool(name="ofp", bufs=BPC) as ofpool,
            tc.tile_pool(name="mid", bufs=pipe + 3 if deep else pipe + 2) as mid,
            tc.tile_pool(name="work", bufs=4 if deep else 3) as work,
            tc.tile_pool(name="cts", bufs=2) as ctspool,
            tc.tile_pool(name="ps_scores", bufs=3, space="PSUM") as ps_scores,
            tc.tile_pool(name="ps_tp", bufs=2, space="PSUM") as ps_tp,
            tc.tile_pool(name="ps_ctx", bufs=2, space="PSUM") as ps_ctx,
            tc.tile_pool(name="ps_out", bufs=1, space="PSUM") as ps_out,
        ):
            # ---- constants ----
            peT_sb = consts.tile([128, pe_terms, HT, TP], mm_dt)
            peTr = peT.rearrange("e (ht p) t -> p e ht t", p=128)
            for e in range(pe_terms):
                nc.sync.dma_start(peT_sb[:, e], peTr[:, e])
            wgT_sb = consts.tile([128, HT, C], mm_dt)
            bg_sb = consts.tile([C, 1], F32)
            ident = consts.tile([128, 128], mm_dt)
            make_identity(nc, ident)

            def load_pf(b):
                pf_sb = pfpool.tile([128, HT, L], mm_dt, tag="pf")
                if dma_mode == "split4":
                    # per-ht DMAs: per-partition runs are contiguous 2KB
                    for ht in range(HT):
                        nc.sync.dma_start(pf_sb[:, ht, :], pfT[b, :, ht, :])
                elif dma_mode in ("single", "2q_big"):
                    # one 1MB dma_start; per-partition run = 8KB contiguous
                    nc.sync.dma_start(pf_sb, pfT[b])
                elif dma_mode == "act_single":
                    nc.scalar.dma_start(pf_sb, pfT[b])
                elif dma_mode == "halves":
                    nc.sync.dma_start(pf_sb[:, :2, :], pfT[b, :, :2, :])
                    nc.sync.dma_start(pf_sb[:, 2:, :], pfT[b, :, 2:, :])
                elif dma_mode == "2q":
                    nc.sync.dma_start(pf_sb[:, :2, :], pfT[b, :, :2, :])
                    nc.scalar.dma_start(pf_sb[:, 2:, :], pfT[b, :, 2:, :])
                elif dma_mode == "3q":
                    nc.sync.dma_start(pf_sb[:, :2, :], pfT[b, :, :2, :])
                    nc.scalar.dma_start(pf_sb[:, 2, :], pfT[b, :, 2, :])
                    nc.gpsimd.dma_start(pf_sb[:, 3, :], pfT[b, :, 3, :])
                return pf_sb

            def load_of(b):
                of_sb = ofpool.tile([128, LT, H], of_dt, tag="of")
                if dma_mode == "split4":
                    for i in range(4):
                        nc.sync.dma_start(
                            of_sb[:, 2 * i : 2 * (i + 1), :], of[b, :, 2 * i : 2 * (i + 1), :]
                        )
                elif dma_mode == "single":
                    nc.sync.dma_start(of_sb, of[b])
                elif dma_mode == "2q_big":
                    nc.scalar.dma_start(of_sb, of[b])
                elif dma_mode == "act_single":
                    nc.scalar.dma_start(of_sb, of[b])
                elif dma_mode == "halves":
                    nc.sync.dma_start(of_sb, of[b])
                elif dma_mode == "2q":
                    nc.sync.dma_start(of_sb[:, :4, :], of[b, :, :4, :])
                    nc.scalar.dma_start(of_sb[:, 4:, :], of[b, :, 4:, :])
                elif dma_mode == "3q":
                    nc.scalar.dma_start(of_sb[:, :4, :], of[b, :, :4, :])
                    nc.gpsimd.dma_start(of_sb[:, 4:, :], of[b, :, 4:, :])
                return of_sb

            def front(i, pf_sb, prevT):
                """mm1(i) with attn-T(i-1) transposes interleaved between the
                matmul streams (each transpose's LDWEIGHTS hides under the
                neighboring N=512 stream: ~366ns/pair vs 440 separate), then
                the softmax chain for i on DVE/ACT. Returns (p, rinv) of i;
                stores pT(i-1) into state."""
                tp_ps = None
                if prevT is not None:
                    p_prev, rinv_prev = prevT
                    tp_ps = ps_tp.tile([128, LT, TP], mm_dt, tag="tp", name="tp")
                    pT_sb = mid.tile([128, LT, TP], mm_dt, tag="pT", name="pT")
                k = 0
                if pf_sb is not None:
                    sc_ps = [
                        ps_scores.tile([TP, 512], F32, tag="scores", name=f"sc{lh}")
                        for lh in range(L // 512)
                    ]
                    for lh in range(L // 512):
                        for ht in range(HT):
                            nc.tensor.matmul(
                                sc_ps[lh],
                                lhsT=peT_sb[:, 0, ht, :],
                                rhs=pf_sb[:, ht, lh * 512 : (lh + 1) * 512],
                                start=(ht == 0),
                                stop=(ht == HT - 1),
                            )
                            if tp_ps is not None and k < LT:
                                nc.tensor.transpose(
                                    tp_ps[:, k, :],
                                    p_prev[:, k * 128 : (k + 1) * 128],
                                    ident[:TP, :TP],
                                )
                                k += 1
                if tp_ps is not None:
                    while k < LT:
                        nc.tensor.transpose(
                            tp_ps[:, k, :], p_prev[:, k * 128 : (k + 1) * 128], ident[:TP, :TP]
                        )
                        k += 1
                    half = LT // 2
                    nc.vector.tensor_copy(pT_sb[:, :half, :], tp_ps[:, :half, :])
                    nc.scalar.copy(pT_sb[:, half:, :], tp_ps[:, half:, :])
                    state[i - 1] = (pT_sb, rinv_prev)

                if pf_sb is None:
                    return None
                m2 = work.tile([TP, 2], F32, tag="m2")
                for lh in range(L // 512):
                    nc.vector.tensor_reduce(m2[:, lh : lh + 1], sc_ps[lh], axis=AX.X, op=OP.max)
                negm = work.tile([TP, 1], F32, tag="negm")
                nc.vector.tensor_reduce(negm, m2, axis=AX.X, op=OP.max, negate=True)
                p_sb = work.tile([TP, L], mm_dt, tag="p")
                s2 = work.tile([TP, 2], F32, tag="s2")
                for lh in range(L // 512):
                    nc.scalar.activation(
                        p_sb[:, lh * 512 : (lh + 1) * 512],
                        sc_ps[lh],
                        AF.Exp,
                        bias=negm,
                        scale=1.0,
                        accum_out=s2[:, lh : lh + 1],
                    )
                ssum = work.tile([TP, 1], F32, tag="ssum")
                nc.vector.tensor_reduce(ssum, s2, axis=AX.X, op=OP.add)
                rinv = mid.tile([TP, 1], F32, tag="rinv")
                nc.vector.reciprocal(rinv, ssum)
                return p_sb, rinv

            def back_mm2(bb, of_sb, pT_sb, rinv):
                """mm2(bb) with ctx-T(bb-1) transposes interleaved; then the
                rinv-scaled PSUM copy-out for bb."""
                prev_ctx = ctxs.pop(bb - 1, None)
                tp_ps = None
                if prev_ctx is not None:
                    tp_ps = ps_tp.tile([128, LT, TP], mm_dt, tag="tp", name="tpc")
                k = 0
                ctx_ps = ps_ctx.tile([TP, H], F32, tag="ctx")
                for lt in range(LT):
                    nc.tensor.matmul(
                        ctx_ps,
                        lhsT=pT_sb[:, lt, :],
                        rhs=of_sb[:, lt, :],
                        start=(lt == 0),
                        stop=(lt == LT - 1),
                    )
                    if tp_ps is not None and k < HT:
                        nc.tensor.transpose(
                            tp_ps[:, k, :],
                            prev_ctx[:, k * 128 : (k + 1) * 128],
                            ident[:TP, :TP],
                        )
                        k += 1
                if tp_ps is not None:
                    g = (bb - 1) // MM3_GRP
                    nc.scalar.copy(cT4[g][:, :, (bb - 1) % MM3_GRP, :], tp_ps[:, :HT, :])
                    if (bb - 1) % MM3_GRP == MM3_GRP - 1:
                        back_mm3(g * MM3_GRP, MM3_GRP, cT4[g])
                ctx_sb = work.tile([TP, H], mm_dt, tag="ctx_sb")
                nc.vector.tensor_scalar_mul(ctx_sb, ctx_ps[:], rinv)
                return ctx_sb

            def final_tp(bb):
                """drain: ctx transpose + copy + mm3 for the last group."""
                prev_ctx = ctxs.pop(bb)
                tp_ps = ps_tp.tile([128, LT, TP], mm_dt, tag="tp", name="tpc")
                for k in range(HT):
                    nc.tensor.transpose(
                        tp_ps[:, k, :], prev_ctx[:, k * 128 : (k + 1) * 128], ident[:TP, :TP]
                    )
                g = bb // MM3_GRP
                nc.scalar.copy(cT4[g][:, :, bb % MM3_GRP, :], tp_ps[:, :HT, :])
                back_mm3(g * MM3_GRP, BPC - g * MM3_GRP, cT4[g])

            def back_mm3(b0, nb, cT4_sb):
                """mm3 + bias + store for batches b0..b0+nb-1."""
                o_ps = ps_out.tile([C, MM3_GRP * TP], F32, tag="o")
                for ht in range(HT):
                    nc.tensor.matmul(
                        o_ps[:, : nb * TP],
                        lhsT=wgT_sb[:, ht, :],
                        rhs=cT4_sb[:, ht, :nb, :],
                        start=(ht == 0),
                        stop=(ht == HT - 1),
                    )
                out_sb = work.tile([C, MM3_GRP, TP], mm_dt, tag="out_sb")
                nc.vector.tensor_scalar_add(
                    out_sb[:, :nb, :],
                    o_ps[:, : nb * TP].rearrange("c (b t) -> c b t", b=nb),
                    bg_sb,
                )
                nc.gpsimd.dma_start(
                    outT[b0 : b0 + nb].rearrange("b c t -> c b t"), out_sb[:, :nb, :]
                )

            state = {}
            ctxs = {}
            cT4 = {}

            nodma_tiles = None
            if only == "nodma":
                pf0_sb = consts.tile([128, HT, L], mm_dt, name="pf0c")
                of0_sb = consts.tile([128, LT, H], of_dt, name="of0c")
                nc.sync.dma_start(pf0_sb, pfT[0])
                nc.sync.dma_start(of0_sb, of[0])
                nodma_tiles = (pf0_sb, of0_sb)

            def body(load_consts):
                if only == "empty":
                    nc.vector.tensor_copy(bg_sb, bg_sb)
                    return
                if only == "pe":
                    # dense independent matmuls: 64 x 512 rows = 32768 PE
                    # cycles -> 13.6us warm / 27.3us cold
                    pf_sb = pfpool.tile([128, HT, L], mm_dt, tag="pf", name="pf")
                    nc.sync.dma_start(pf_sb, pfT[0])
                    for i in range(64):
                        sc = ps_scores.tile([TP, 512], F32, tag="scores", name="sc")
                        nc.tensor.matmul(
                            sc,
                            lhsT=peT_sb[:, 0, i % HT, :],
                            rhs=pf_sb[:, i % HT, (i % 2) * 512 : (i % 2 + 1) * 512],
                            start=True,
                            stop=True,
                        )
                    return
                if only == "pe_grp":
                    # 16 groups of 4 accumulating MMs (64 MMs, N=512)
                    pf_sb = pfpool.tile([128, HT, L], mm_dt, tag="pf", name="pf")
                    nc.sync.dma_start(pf_sb, pfT[0])
                    for g in range(16):
                        sc = ps_scores.tile([TP, 512], F32, tag="scores", name="sc")
                        for j in range(4):
                            nc.tensor.matmul(
                                sc,
                                lhsT=peT_sb[:, 0, j, :],
                                rhs=pf_sb[:, j, (g % 2) * 512 : (g % 2 + 1) * 512],
                                start=(j == 0),
                                stop=(j == 3),
                            )
                    return
                if only == "pe_tp":
                    # 64 transposes of [TP,128] -> overhead probe
                    p_sb = work.tile([TP, L], mm_dt, tag="p", name="p")
                    nc.sync.dma_start(p_sb, pfT[0, :TP, 0, :])
                    for i in range(64):
                        tp_ps = ps_tp.tile([128, TP], mm_dt, tag="tp", name="tp")
                        nc.tensor.transpose(
                            tp_ps, p_sb[:, (i % 8) * 128 : (i % 8 + 1) * 128], ident[:TP, :TP]
                        )
                    return
                if only == "pe_n1024":
                    # 32 MMs with N=1024 out (2 PSUM banks) - legality probe
                    pf_sb = pfpool.tile([128, HT, L], mm_dt, tag="pf", name="pf")
                    nc.sync.dma_start(pf_sb, pfT[0])
                    for i in range(32):
                        sc = ps_scores.tile([TP, 1024], F32, tag="sc1k", name="sc")
                        nc.tensor.matmul(
                            sc,
                            lhsT=peT_sb[:, 0, i % HT, :],
                            rhs=pf_sb[:, i % HT, :],
                            start=True,
                            stop=True,
                        )
                    return
                if only == "pe_acc":
                    # 64 MMs N=512 accumulating into ONE psum tile: no
                    # per-MM WAR semaphores, one group
                    pf_sb = pfpool.tile([128, HT, L], mm_dt, tag="pf", name="pf")
                    nc.sync.dma_start(pf_sb, pfT[0])
                    sc = ps_scores.tile([TP, 512], F32, tag="scores", name="sc")
                    for i in range(64):
                        nc.tensor.matmul(
                            sc,
                            lhsT=peT_sb[:, 0, i % HT, :],
                            rhs=pf_sb[:, i % HT, (i % 2) * 512 : (i % 2 + 1) * 512],
                            start=(i == 0),
                            stop=(i == 63),
                        )
                    nc.vector.tensor_reduce(
                        work.tile([TP, 1], F32, tag="m2", name="m2"), sc, axis=AX.X, op=OP.max
                    )
                    return
                if only == "pe_same":
                    # 64 MMs N=512, all with the SAME stationary operand
                    pf_sb = pfpool.tile([128, HT, L], mm_dt, tag="pf", name="pf")
                    nc.sync.dma_start(pf_sb, pfT[0])
                    for i in range(64):
                        sc = ps_scores.tile([TP, 512], F32, tag="scores", name="sc")
                        nc.tensor.matmul(
                            sc,
                            lhsT=peT_sb[:, 0, 0, :],
                            rhs=pf_sb[:, i % HT, (i % 2) * 512 : (i % 2 + 1) * 512],
                            start=True,
                            stop=True,
                        )
                    return
                if only == "pe_mix":
                    # 32 MMs N=512 alternating with 32 transposes
                    pf_sb = pfpool.tile([128, HT, L], mm_dt, tag="pf", name="pf")
                    nc.sync.dma_start(pf_sb, pfT[0])
                    p_sb = work.tile([TP, L], mm_dt, tag="p", name="p")
                    nc.sync.dma_start(p_sb, pfT[0, :TP, 0, :])
                    for i in range(32):
                        sc = ps_scores.tile([TP, 512], F32, tag="scores", name="sc")
                        nc.tensor.matmul(
                            sc,
                            lhsT=peT_sb[:, 0, i % HT, :],
                            rhs=pf_sb[:, i % HT, (i % 2) * 512 : (i % 2 + 1) * 512],
                            start=True,
                            stop=True,
                        )
                        tp_ps = ps_tp.tile([128, TP], mm_dt, tag="tp", name="tp")
                        nc.tensor.transpose(
                            tp_ps, p_sb[:, (i % 8) * 128 : (i % 8 + 1) * 128], ident[:TP, :TP]
                        )
                    return
                # ---- DMA queue order: JIT arrivals ----
                pf_tiles = {}
                of_tiles = {}
                if only == "nodma":
                    for b in range(BPC):
                        pf_tiles[b] = nodma_tiles[0]
                        of_tiles[b] = nodma_tiles[1]
                else:
                    for b in range(min(pipe, BPC)):
                        pf_tiles[b] = load_pf(b)
                    if load_consts:
                        nc.sync.dma_start(wgT_sb, wgT.rearrange("(ht p) c -> p ht c", p=128))
                        nc.sync.dma_start(bg_sb, bg)
                    for b in range(pipe, BPC):
                        pf_tiles[b] = load_pf(b)
                        of_tiles[b - pipe] = load_of(b - pipe)
                    for b in range(max(BPC - pipe, 0), BPC):
                        of_tiles[b] = load_of(b)
                if only == "dma":
                    return

                # ---- compute: PE order per step i is
                #   [mm1(i) ⊗ attn-T(i-1)] | [mm2(i-pipe) ⊗ ctx-T(i-pipe-1), mm3]
                # (⊗ = transposes interleaved between matmul streams so their
                # LDWEIGHTS hide; the softmax chain of batch i hides under
                # the back-half matmuls). ----
                state.clear()
                ctxs.clear()
                cT4.clear()
                prev = None
                for i in range(BPC + pipe):
                    if i < BPC:
                        prev = front(i, pf_tiles.pop(i), prev)
                    elif i == BPC:
                        front(i, None, prev)
                        prev = None
                    if i >= pipe:
                        bb = i - pipe
                        if bb % MM3_GRP == 0:
                            cT4[bb // MM3_GRP] = ctspool.tile(
                                [128, HT, MM3_GRP, TP], mm_dt, tag="cT4", name="cT4"
                            )
                        ctxs[bb] = back_mm2(bb, of_tiles.pop(bb), *state.pop(bb))
                final_tp(BPC - 1)

            if timing:
                # unroll several reps per For_i iteration: the loop's
                # all-engine barrier idles PE long enough to re-engage the
                # HAM clock throttle (1.2 GHz), so amortize it to ~1% and
                # measure warm steady-state throughput like an unrolled NEFF
                assert hw_loop % unroll == 0
                nc.sync.dma_start(wgT_sb, wgT.rearrange("(ht p) c -> p ht c", p=128))
                nc.sync.dma_start(bg_sb, bg)
                with tc.For_i(0, hw_loop // unroll, 1):
                    for _u in range(unroll):
                        body(load_consts=False)
                nc.gpsimd.dma_start(tok, bg_sb)
            else:
                for _rep in range(repeats):
                    body(load_consts=(_rep == 0))

    nc.compile()
    return nc


_NC = None


def _get_nc():
    global _NC
    if _NC is None:
        _NC = build_nc()
    return _NC


def make_in_maps(position_fmap, origin_fmap, pos_emb, W_gen, b_gen, np_dt=NP_DT, pe_terms=PE_TERMS, of_np=OF_NP):
    """Host-side sharding + layout prep. Returns list of per-core input dicts."""
    pf = np.asarray(position_fmap, dtype=np.float32)
    of = np.asarray(origin_fmap, dtype=np.float32)
    pe = np.asarray(pos_emb, dtype=np.float32)
    wg = np.asarray(W_gen, dtype=np.float32)
    bgv = np.asarray(b_gen, dtype=np.float32)

    # [B, L, H] -> [B, H, L] -> [B, 128, HT, L]  (partition-major, h = ht*128 + p)
    pfT = np.ascontiguousarray(
        pf.transpose(0, 2, 1).reshape(B, HT, 128, L).transpose(0, 2, 1, 3)
    ).astype(np_dt)
    # [B, L, H] -> [B, 128, LT, H]  (partition-major tiling, l = lt*128 + p)
    of_c = np.ascontiguousarray(
        of.reshape(B, LT, 128, H).transpose(0, 2, 1, 3)
    ).astype(of_np)

    peT_f32 = np.zeros((H, TP), dtype=np.float32)
    peT_f32[:, :T] = pe.T
    terms = []
    resid = peT_f32
    for _ in range(pe_terms):
        t = resid.astype(np_dt)
        terms.append(t)
        resid = resid - t.astype(np.float32)
    peT = np.ascontiguousarray(np.stack(terms, axis=0))  # [pe_terms, H, TP]

    wgT = np.ascontiguousarray(wg.T).astype(np_dt)
    bg2 = np.ascontiguousarray(bgv.reshape(C, 1)).astype(np.float32)

    in_maps = []
    for i in range(NCORES):
        sl = slice(i * BPC, (i + 1) * BPC)
        in_maps.append(
            {
                "pfT": pfT[sl],
                "of": of_c[sl],
                "peT": peT,
                "wgT": wgT,
                "bg": bg2,
            }
        )
    return in_maps


def kernel(position_fmap, origin_fmap, pos_emb, W_gen, b_gen):
    nc = _get_nc()
    in_maps = make_in_maps(position_fmap, origin_fmap, pos_emb, W_gen, b_gen)
    res = run_bass_kernel_spmd(nc, in_maps, core_ids=list(range(NCORES)))
    outs = [r["outT"] for r in res.results]  # each [BPC, C, TP]
    out = np.concatenate(outs, axis=0)[:, :, :T]  # [B, C, T]
    return np.ascontiguousarray(out.transpose(0, 2, 1)).astype(np.float32)


# revision 44
# speedup vs baseline: 1.1074x; 1.0434x over previous
"""Bass/Tile kernel for nn_AttnModule (sparse_attention).

Reference computation (per batch b):
    scores  = pos_emb @ position_fmap[b].T          # [T, L]
    attn    = softmax(scores, axis=-1)              # softmax over L
    context = attn @ origin_fmap[b]                 # [T, H]
    out     = context @ W_gen.T + b_gen             # [T, C]

Sharding: pure data parallel over batch B=64 -> 8 cores x 8 batches.

Measured cost model (this hw, via differential For_i timing):
  - every PE matmul instruction costs ~59ns on top of its N-cycle
    stream @2.4GHz, regardless of grouping/stationary reuse; a
    transpose costs ~168ns standalone but only ~94ns marginal when
    interleaved between N=512 matmul streams.
  - HBM DMA sustains ~340GB/s per core only with big (>=0.5MB)
    dma_start entries on one queue; 4-way splits drop to ~230GB/s
    and multi-queue spreading is slower.
  - the PE HAM clock throttle (1.2GHz cold / 2.4GHz warm, 3.4us
    free-running windows) makes idle-interleaved schedules ~2x slow.

v2 design vs the hi/lo baseline (65.3us -> 47.2us same-method):
  - pos_emb single fp16 term (mm1 8192 -> 4096 cyc/batch); logit
    rounding error from fp16 pf dominates anyway and stays ~1.4e-2.
  - origin_fmap streamed as fp8 E3M4 (4 mantissa bits, data ~N(0,1)):
    halves the of DMA bytes; mm2 takes the fp8 moving operand at fp16
    rate (mixed-dtype matmul), PSUM accumulates fp32.
  - TP=112 (T=100 padded) instead of 128 trims transpose/mm3 streams.
  - mm3 batched over groups of 4 batches (one [C,4*TP] PSUM tile) to
    amortize the wgT weight load.
  - software-pipelined schedule: step i emits [mm1(i) with attn-T(i-1)
    transposes interleaved] then [mm2(i-PIPE) with ctx-T(i-PIPE-1)
    interleaved + mm3]; the DMA queue is ordered pf0..pf2, (pf3,of0),
    (pf4,of1).. so arrivals are just-in-time for both phases.
  - per-batch PE floor reached: 16 stream MMs + 12 interleaved
    transposes + mm3/4 = ~5.3us/batch incl. instruction overheads.

Layout choices (host side prep is free):
  - position_fmap shipped pre-transposed per batch: pfT [B, 128, HT, L]
  - pos_emb shipped transposed fp16, T zero-padded to TP: peT [1, H, TP]
  - origin_fmap shipped l-tiled fp8e3: of [B, 128, LT, H]
  - W_gen shipped transposed: wgT [H, C]
  - output produced as [B, C, TP] fp16, transposed back on host.
"""

import numpy as np
import ml_dtypes

import concourse.mybir as mybir
import concourse.tile as tile
from concourse import bacc
from concourse.bass_utils import run_bass_kernel_spmd
from concourse.masks import make_identity

B, L, H, T, C = 64, 1024, 512, 100, 97
TP = 112
NCORES = 8
BPC = B // NCORES  # batches per core

HT = H // 128  # 4 h-tiles
LT = L // 128  # 8 l-tiles

F32 = mybir.dt.float32
AF = mybir.ActivationFunctionType
AX = mybir.AxisListType
OP = mybir.AluOpType

MM_DT = mybir.dt.float16
NP_DT = np.float16
OF_DT = mybir.dt.float8e3
OF_NP = ml_dtypes.float8_e3m4
PE_TERMS = 1
PIPE = 3  # back(b-PIPE) interleaves with front(b)
MM3_GRP = 4


def build_nc(mm_dt=MM_DT, of_dt=OF_DT, pe_terms=PE_TERMS, repeats=1, pipe=PIPE, hw_loop=None, only=None, dma_mode="single", unroll=8, order_swap=False, deep=False):
    """hw_loop=N wraps the body in a For_i hardware loop (timing builds):
    outT becomes an Internal scratch and a tiny token is the only external
    output, so per-call wire traffic is negligible and device time
    dominates."""
    timing = hw_loop is not None
    nc = bacc.Bacc(None, target_bir_lowering=False, debug=False)

    pfT = nc.dram_tensor("pfT", [BPC, 128, HT, L], mm_dt, kind="ExternalInput").ap()
    of = nc.dram_tensor("of", [BPC, 128, LT, H], of_dt, kind="ExternalInput").ap()
    peT = nc.dram_tensor("peT", [pe_terms, H, TP], mm_dt, kind="ExternalInput").ap()
    wgT = nc.dram_tensor("wgT", [H, C], mm_dt, kind="ExternalInput").ap()
    bg = nc.dram_tensor("bg", [C, 1], F32, kind="ExternalInput").ap()
    if timing:
        outT = nc.dram_tensor("outT", [BPC, C, TP], mm_dt, kind="Internal").ap()
        tok = nc.dram_tensor("tok", [C, 1], F32, kind="ExternalOutput").ap()
    else:
        outT = nc.dram_tensor("outT", [BPC, C, TP], mm_dt, kind="ExternalOutput").ap()

    with tile.TileContext(nc) as tc:
        with (
            tc.tile_pool(name="consts", bufs=1) as consts,
            tc.tile_pool(name="pf", bufs=BPC) as pfpool,
            tc.tile_pool(name="ofp", bufs=BPC) as ofpool,
            tc.tile_pool(name="mid", bufs=pipe + 3 if deep else pipe + 2) as mid,
            tc.tile_pool(name="work", bufs=4 if deep else 3) as work,
            tc.tile_pool(name="cts", bufs=2) as ctspool,
            tc.tile_pool(name="ps_scores", bufs=3, space="PSUM") as ps_scores,
            tc.tile_pool(name="ps_tp", bufs=2, space="PSUM") as ps_tp,
            tc.tile_pool(name="ps_ctx", bufs=2, space="PSUM") as ps_ctx,
            tc.tile_pool(name="ps_out", bufs=1, space="PSUM") as ps_out,
        ):
            # ---- constants ----
            peT_sb = consts.tile([128, pe_terms, HT, TP], mm_dt)
            peTr = peT.rearrange("e (ht p) t -> p e ht t", p=128)
            for e in range(pe_terms):
                nc.sync.dma_start(peT_sb[:, e], peTr[:, e])
            wgT_sb = consts.tile([128, HT, C], mm_dt)
            bg_sb = consts.tile([C, 1], F32)
            ident = consts.tile([128, 128], mm_dt)
            make_identity(nc, ident)

            def load_pf(b):
                pf_sb = pfpool.tile([128, HT, L], mm_dt, tag="pf")
                if dma_mode == "split4":
                    # per-ht DMAs: per-partition runs are contiguous 2KB
                    for ht in range(HT):
                        nc.sync.dma_start(pf_sb[:, ht, :], pfT[b, :, ht, :])
                elif dma_mode in ("single", "2q_big"):
                    # one 1MB dma_start; per-partition run = 8KB contiguous
                    nc.sync.dma_start(pf_sb, pfT[b])
                elif dma_mode == "act_single":
                    nc.scalar.dma_start(pf_sb, pfT[b])
                elif dma_mode == "halves":
                    nc.sync.dma_start(pf_sb[:, :2, :], pfT[b, :, :2, :])
                    nc.sync.dma_start(pf_sb[:, 2:, :], pfT[b, :, 2:, :])
                elif dma_mode == "2q":
                    nc.sync.dma_start(pf_sb[:, :2, :], pfT[b, :, :2, :])
                    nc.scalar.dma_start(pf_sb[:, 2:, :], pfT[b, :, 2:, :])
                elif dma_mode == "3q":
                    nc.sync.dma_start(pf_sb[:, :2, :], pfT[b, :, :2, :])
                    nc.scalar.dma_start(pf_sb[:, 2, :], pfT[b, :, 2, :])
                    nc.gpsimd.dma_start(pf_sb[:, 3, :], pfT[b, :, 3, :])
                return pf_sb

            def load_of(b):
                of_sb = ofpool.tile([128, LT, H], of_dt, tag="of")
                if dma_mode == "split4":
                    for i in range(4):
                        nc.sync.dma_start(
                            of_sb[:, 2 * i : 2 * (i + 1), :], of[b, :, 2 * i : 2 * (i + 1), :]
                        )
                elif dma_mode == "single":
                    nc.sync.dma_start(of_sb, of[b])
                elif dma_mode == "2q_big":
                    nc.scalar.dma_start(of_sb, of[b])
                elif dma_mode == "act_single":
                    nc.scalar.dma_start(of_sb, of[b])
                elif dma_mode == "halves":
                    nc.sync.dma_start(of_sb, of[b])
                elif dma_mode == "2q":
                    nc.sync.dma_start(of_sb[:, :4, :], of[b, :, :4, :])
                    nc.scalar.dma_start(of_sb[:, 4:, :], of[b, :, 4:, :])
                elif dma_mode == "3q":
                    nc.scalar.dma_start(of_sb[:, :4, :], of[b, :, :4, :])
                    nc.gpsimd.dma_start(of_sb[:, 4:, :], of[b, :, 4:, :])
                return of_sb

            def front(i, pf_sb, prevT):
                """mm1(i) with attn-T(i-1) transposes interleaved between the
                matmul streams (each transpose's LDWEIGHTS hides under the
                neighboring N=512 stream: ~366ns/pair vs 440 separate), then
                the softmax chain for i on DVE/ACT. Returns (p, rinv) of i;
                stores pT(i-1) into state."""
                tp_ps = None
                if prevT is not None:
                    p_prev, rinv_prev = prevT
                    tp_ps = ps_tp.tile([128, LT, TP], mm_dt, tag="tp", name="tp")
                    pT_sb = mid.tile([128, LT, TP], mm_dt, tag="pT", name="pT")
                k = 0
                if pf_sb is not None:
                    sc_ps = [
                        ps_scores.tile([TP, 512], F32, tag="scores", name=f"sc{lh}")
                        for lh in range(L // 512)
                    ]
                    for lh in range(L // 512):
                        for ht in range(HT):
                            nc.tensor.matmul(
                                sc_ps[lh],
                                lhsT=peT_sb[:, 0, ht, :],
                                rhs=pf_sb[:, ht, lh * 512 : (lh + 1) * 512],
                                start=(ht == 0),
                                stop=(ht == HT - 1),
                            )
                            if tp_ps is not None and k < LT:
                                nc.tensor.transpose(
                                    tp_ps[:, k, :],
                                    p_prev[:, k * 128 : (k + 1) * 128],
                                    ident[:TP, :TP],
                                )
                                k += 1
                if tp_ps is not None:
                    while k < LT:
                        nc.tensor.transpose(
                            tp_ps[:, k, :], p_prev[:, k * 128 : (k + 1) * 128], ident[:TP, :TP]
                        )
                        k += 1
                    half = LT // 2
                    nc.vector.tensor_copy(pT_sb[:, :half, :], tp_ps[:, :half, :])
                    nc.scalar.copy(pT_sb[:, half:, :], tp_ps[:, half:, :])
                    state[i - 1] = (pT_sb, rinv_prev)

                if pf_sb is None:
                    return None
                m2 = work.tile([TP, 2], F32, tag="m2")
                for lh in range(L // 512):
                    nc.vector.tensor_reduce(m2[:, lh : lh + 1], sc_ps[lh], axis=AX.X, op=OP.max)
                negm = work.tile([TP, 1], F32, tag="negm")
                nc.vector.tensor_reduce(negm, m2, axis=AX.X, op=OP.max, negate=True)
                p_sb = work.tile([TP, L], mm_dt, tag="p")
                s2 = work.tile([TP, 2], F32, tag="s2")
                for lh in range(L // 512):
                    nc.scalar.activation(
                        p_sb[:, lh * 512 : (lh + 1) * 512],
                        sc_ps[lh],
                        AF.Exp,
                        bias=negm,
                        scale=1.0,
                        accum_out=s2[:, lh : lh + 1],
                    )
                ssum = work.tile([TP, 1], F32, tag="ssum")
                nc.vector.tensor_reduce(ssum, s2, axis=AX.X, op=OP.add)
                rinv = mid.tile([TP, 1], F32, tag="rinv")
                nc.vector.reciprocal(rinv, ssum)
                return p_sb, rinv

            def back_mm2(bb, of_sb, pT_sb, rinv):
                """mm2(bb) with ctx-T(bb-1) transposes interleaved; then the
                rinv-scaled PSUM copy-out for bb."""
                prev_ctx = ctxs.pop(bb - 1, None)
                tp_ps = None
                if prev_ctx is not None:
                    tp_ps = ps_tp.tile([128, LT, TP], mm_dt, tag="tp", name="tpc")
                k = 0
                ctx_ps = ps_ctx.tile([TP, H], F32, tag="ctx")
                for lt in range(LT):
                    nc.tensor.matmul(
                        ctx_ps,
                        lhsT=pT_sb[:, lt, :],
                        rhs=of_sb[:, lt, :],
                        start=(lt == 0),
                        stop=(lt == LT - 1),
                    )
                    if tp_ps is not None and k < HT:
                        nc.tensor.transpose(
                            tp_ps[:, k, :],
                            prev_ctx[:, k * 128 : (k + 1) * 128],
                            ident[:TP, :TP],
                        )
                        k += 1
                if tp_ps is not None:
                    g = (bb - 1) // MM3_GRP
                    nc.scalar.copy(cT4[g][:, :, (bb - 1) % MM3_GRP, :], tp_ps[:, :HT, :])
                    if (bb - 1) % MM3_GRP == MM3_GRP - 1:
                        back_mm3(g * MM3_GRP, MM3_GRP, cT4[g])
                ctx_sb = work.tile([TP, H], mm_dt, tag="ctx_sb")
                nc.vector.tensor_scalar_mul(ctx_sb, ctx_ps[:], rinv)
                return ctx_sb

            def final_tp(bb):
                """drain: ctx transpose + copy + mm3 for the last group."""
                prev_ctx = ctxs.pop(bb)
                tp_ps = ps_tp.tile([128, LT, TP], mm_dt, tag="tp", name="tpc")
                for k in range(HT):
                    nc.tensor.transpose(
                        tp_ps[:, k, :], prev_ctx[:, k * 128 : (k + 1) * 128], ident[:TP, :TP]
                    )
                g = bb // MM3_GRP
                nc.scalar.copy(cT4[g][:, :, bb % MM3_GRP, :], tp_ps[:, :HT, :])
                back_mm3(g * MM3_GRP, BPC - g * MM3_GRP, cT4[g])

            def back_mm3(b0, nb, cT4_sb):
                """mm3 + bias + store for batches b0..b0+nb-1."""
                o_ps = ps_out.tile([C, MM3_GRP * TP], F32, tag="o")
                for ht in range(HT):
                    nc.tensor.matmul(
                        o_ps[:, : nb * TP],
                        lhsT=wgT_sb[:, ht, :],
                        rhs=cT4_sb[:, ht, :nb, :],
                        start=(ht == 0),
                        stop=(ht == HT - 1),
                    )
                out_sb = work.tile([C, MM3_GRP, TP], mm_dt, tag="out_sb")
                nc.vector.tensor_scalar_add(
                    out_sb[:, :nb, :],
                    o_ps[:, : nb * TP].rearrange("c (b t) -> c b t", b=nb),
                    bg_sb,
                )
                nc.gpsimd.dma_start(
                    outT[b0 : b0 + nb].rearrange("b c t -> c b t"), out_sb[:, :nb, :]
                )

            state = {}
            ctxs = {}
            cT4 = {}

            nodma_tiles = None
            if only == "nodma":
                pf0_sb = consts.tile([128, HT, L], mm_dt, name="pf0c")
                of0_sb = consts.tile([128, LT, H], of_dt, name="of0c")
                nc.sync.dma_start(pf0_sb, pfT[0])
                nc.sync.dma_start(of0_sb, of[0])
                nodma_tiles = (pf0_sb, of0_sb)

            def body(load_consts, warm=False):
                # (HAM warmup via throwaway matmuls was tried here and
                # measured 2.4us/rep SLOWER - do not re-add.)
                if only == "empty":
                    nc.vector.tensor_copy(bg_sb, bg_sb)
                    return
                if only == "pe":
                    # dense independent matmuls: 64 x 512 rows = 32768 PE
                    # cycles -> 13.6us warm / 27.3us cold
                    pf_sb = pfpool.tile([128, HT, L], mm_dt, tag="pf", name="pf")
                    nc.sync.dma_start(pf_sb, pfT[0])
                    for i in range(64):
                        sc = ps_scores.tile([TP, 512], F32, tag="scores", name="sc")
                        nc.tensor.matmul(
                            sc,
                            lhsT=peT_sb[:, 0, i % HT, :],
                            rhs=pf_sb[:, i % HT, (i % 2) * 512 : (i % 2 + 1) * 512],
                            start=True,
                            stop=True,
                        )
                    return
                if only == "pe_grp":
                    # 16 groups of 4 accumulating MMs (64 MMs, N=512)
                    pf_sb = pfpool.tile([128, HT, L], mm_dt, tag="pf", name="pf")
                    nc.sync.dma_start(pf_sb, pfT[0])
                    for g in range(16):
                        sc = ps_scores.tile([TP, 512], F32, tag="scores", name="sc")
                        for j in range(4):
                            nc.tensor.matmul(
                                sc,
                                lhsT=peT_sb[:, 0, j, :],
                                rhs=pf_sb[:, j, (g % 2) * 512 : (g % 2 + 1) * 512],
                                start=(j == 0),
                                stop=(j == 3),
                            )
                    return
                if only == "pe_tp":
                    # 64 transposes of [TP,128] -> overhead probe
                    p_sb = work.tile([TP, L], mm_dt, tag="p", name="p")
                    nc.sync.dma_start(p_sb, pfT[0, :TP, 0, :])
                    for i in range(64):
                        tp_ps = ps_tp.tile([128, TP], mm_dt, tag="tp", name="tp")
                        nc.tensor.transpose(
                            tp_ps, p_sb[:, (i % 8) * 128 : (i % 8 + 1) * 128], ident[:TP, :TP]
                        )
                    return
                if only == "pe_n1024":
                    # 32 MMs with N=1024 out (2 PSUM banks) - legality probe
                    pf_sb = pfpool.tile([128, HT, L], mm_dt, tag="pf", name="pf")
                    nc.sync.dma_start(pf_sb, pfT[0])
                    for i in range(32):
                        sc = ps_scores.tile([TP, 1024], F32, tag="sc1k", name="sc")
                        nc.tensor.matmul(
                            sc,
                            lhsT=peT_sb[:, 0, i % HT, :],
                            rhs=pf_sb[:, i % HT, :],
                            start=True,
                            stop=True,
                        )
                    return
                if only == "pe_acc":
                    # 64 MMs N=512 accumulating into ONE psum tile: no
                    # per-MM WAR semaphores, one group
                    pf_sb = pfpool.tile([128, HT, L], mm_dt, tag="pf", name="pf")
                    nc.sync.dma_start(pf_sb, pfT[0])
                    sc = ps_scores.tile([TP, 512], F32, tag="scores", name="sc")
                    for i in range(64):
                        nc.tensor.matmul(
                            sc,
                            lhsT=peT_sb[:, 0, i % HT, :],
                            rhs=pf_sb[:, i % HT, (i % 2) * 512 : (i % 2 + 1) * 512],
                            start=(i == 0),
                            stop=(i == 63),
                        )
                    nc.vector.tensor_reduce(
                        work.tile([TP, 1], F32, tag="m2", name="m2"), sc, axis=AX.X, op=OP.max
                    )
                    return
                if only == "pe_same":
                    # 64 MMs N=512, all with the SAME stationary operand
                    pf_sb = pfpool.tile([128, HT, L], mm_dt, tag="pf", name="pf")
                    nc.sync.dma_start(pf_sb, pfT[0])
                    for i in range(64):
                        sc = ps_scores.tile([TP, 512], F32, tag="scores", name="sc")
                        nc.tensor.matmul(
                            sc,
                            lhsT=peT_sb[:, 0, 0, :],
                            rhs=pf_sb[:, i % HT, (i % 2) * 512 : (i % 2 + 1) * 512],
                            start=True,
                            stop=True,
                        )
                    return
                if only == "pe_mix":
                    # 32 MMs N=512 alternating with 32 transposes
                    pf_sb = pfpool.tile([128, HT, L], mm_dt, tag="pf", name="pf")
                    nc.sync.dma_start(pf_sb, pfT[0])
                    p_sb = work.tile([TP, L], mm_dt, tag="p", name="p")
                    nc.sync.dma_start(p_sb, pfT[0, :TP, 0, :])
                    for i in range(32):
                        sc = ps_scores.tile([TP, 512], F32, tag="scores", name="sc")
                        nc.tensor.matmul(
                            sc,
                            lhsT=peT_sb[:, 0, i % HT, :],
                            rhs=pf_sb[:, i % HT, (i % 2) * 512 : (i % 2 + 1) * 512],
                            start=True,
                            stop=True,
                        )
                        tp_ps = ps_tp.tile([128, TP], mm_dt, tag="tp", name="tp")
                        nc.tensor.transpose(
                            tp_ps, p_sb[:, (i % 8) * 128 : (i % 8 + 1) * 128], ident[:TP, :TP]
                        )
                    return
                # ---- DMA queue order: JIT arrivals ----
                pf_tiles = {}
                of_tiles = {}
                if only == "nodma":
                    for b in range(BPC):
                        pf_tiles[b] = nodma_tiles[0]
                        of_tiles[b] = nodma_tiles[1]
                else:
                    for b in range(min(pipe, BPC)):
                        pf_tiles[b] = load_pf(b)
                    if load_consts:
                        nc.sync.dma_start(wgT_sb, wgT.rearrange("(ht p) c -> p ht c", p=128))
                        nc.sync.dma_start(bg_sb, bg)
                    for b in range(pipe, BPC):
                        pf_tiles[b] = load_pf(b)
                        of_tiles[b - pipe] = load_of(b - pipe)
                    for b in range(max(BPC - pipe, 0), BPC):
                        of_tiles[b] = load_of(b)
                if only == "dma":
                    return

                # ---- compute: PE order per step i is
                #   [mm1(i) ⊗ attn-T(i-1)] | [mm2(i-pipe) ⊗ ctx-T(i-pipe-1), mm3]
                # (⊗ = transposes interleaved between matmul streams so their
                # LDWEIGHTS hide; the softmax chain of batch i hides under
                # the back-half matmuls). ----
                state.clear()
                ctxs.clear()
                cT4.clear()
                prev = None
                for i in range(BPC + pipe):
                    if i < BPC:
                        prev = front(i, pf_tiles.pop(i), prev)
                    elif i == BPC:
                        front(i, None, prev)
                        prev = None
                    if i >= pipe:
                        bb = i - pipe
                        if bb % MM3_GRP == 0:
                            cT4[bb // MM3_GRP] = ctspool.tile(
                                [128, HT, MM3_GRP, TP], mm_dt, tag="cT4", name="cT4"
                            )
                        ctxs[bb] = back_mm2(bb, of_tiles.pop(bb), *state.pop(bb))
                final_tp(BPC - 1)

            if timing:
                # unroll several reps per For_i iteration: the loop's
                # all-engine barrier idles PE long enough to re-engage the
                # HAM clock throttle (1.2 GHz), so amortize it to ~1% and
                # measure warm steady-state throughput like an unrolled NEFF
                assert hw_loop % unroll == 0
                nc.sync.dma_start(wgT_sb, wgT.rearrange("(ht p) c -> p ht c", p=128))
                nc.sync.dma_start(bg_sb, bg)
                with tc.For_i(0, hw_loop // unroll, 1):
                    for _u in range(unroll):
                        body(load_consts=False)
                nc.gpsimd.dma_start(tok, bg_sb)
            else:
                for _rep in range(repeats):
                    body(load_consts=(_rep == 0))

    nc.compile()
    return nc


_NC = None


def _get_nc():
    global _NC
    if _NC is None:
        _NC = build_nc()
    return _NC


def make_in_maps(position_fmap, origin_fmap, pos_emb, W_gen, b_gen, np_dt=NP_DT, pe_terms=PE_TERMS, of_np=OF_NP):
    """Host-side sharding + layout prep. Returns list of per-core input dicts."""
    pf = np.asarray(position_fmap, dtype=np.float32)
    of = np.asarray(origin_fmap, dtype=np.float32)
    pe = np.asarray(pos_emb, dtype=np.float32)
    wg = np.asarray(W_gen, dtype=np.float32)
    bgv = np.asarray(b_gen, dtype=np.float32)

    # [B, L, H] -> [B, H, L] -> [B, 128, HT, L]  (partition-major, h = ht*128 + p)
    pfT = np.ascontiguousarray(
        pf.transpose(0, 2, 1).reshape(B, HT, 128, L).transpose(0, 2, 1, 3)
    ).astype(np_dt)
    # [B, L, H] -> [B, 128, LT, H]  (partition-major tiling, l = lt*128 + p)
    of_c = np.ascontiguousarray(
        of.reshape(B, LT, 128, H).transpose(0, 2, 1, 3)
    ).astype(of_np)

    peT_f32 = np.zeros((H, TP), dtype=np.float32)
    peT_f32[:, :T] = pe.T
    terms = []
    resid = peT_f32
    for _ in range(pe_terms):
        t = resid.astype(np_dt)
        terms.append(t)
        resid = resid - t.astype(np.float32)
    peT = np.ascontiguousarray(np.stack(terms, axis=0))  # [pe_terms, H, TP]

    wgT = np.ascontiguousarray(wg.T).astype(np_dt)
    bg2 = np.ascontiguousarray(bgv.reshape(C, 1)).astype(np.float32)

    in_maps = []
    for i in range(NCORES):
        sl = slice(i * BPC, (i + 1) * BPC)
        in_maps.append(
            {
                "pfT": pfT[sl],
                "of": of_c[sl],
                "peT": peT,
                "wgT": wgT,
                "bg": bg2,
            }
        )
    return in_maps


def kernel(position_fmap, origin_fmap, pos_emb, W_gen, b_gen):
    nc = _get_nc()
    in_maps = make_in_maps(position_fmap, origin_fmap, pos_emb, W_gen, b_gen)
    res = run_bass_kernel_spmd(nc, in_maps, core_ids=list(range(NCORES)))
    outs = [r["outT"] for r in res.results]  # each [BPC, C, TP]
    out = np.concatenate(outs, axis=0)[:, :, :T]  # [B, C, T]
    return np.ascontiguousarray(out.transpose(0, 2, 1)).astype(np.float32)


# revision 45
# speedup vs baseline: 1.1074x; 1.0000x over previous
"""Bass/Tile kernel for nn_AttnModule (sparse_attention).

Reference computation (per batch b):
    scores  = pos_emb @ position_fmap[b].T          # [T, L]
    attn    = softmax(scores, axis=-1)              # softmax over L
    context = attn @ origin_fmap[b]                 # [T, H]
    out     = context @ W_gen.T + b_gen             # [T, C]

Sharding: pure data parallel over batch B=64 -> 8 cores x 8 batches.

Measured cost model (this hw, via differential For_i timing):
  - every PE matmul instruction costs ~59ns on top of its N-cycle
    stream @2.4GHz, regardless of grouping/stationary reuse; a
    transpose costs ~168ns standalone but only ~94ns marginal when
    interleaved between N=512 matmul streams.
  - HBM DMA sustains ~340GB/s per core only with big (>=0.5MB)
    dma_start entries on one queue; 4-way splits drop to ~230GB/s
    and multi-queue spreading is slower.
  - the PE HAM clock throttle (1.2GHz cold / 2.4GHz warm, 3.4us
    free-running windows) makes idle-interleaved schedules ~2x slow.

v2 design vs the hi/lo baseline (65.3us -> 47.2us same-method):
  - pos_emb single fp16 term (mm1 8192 -> 4096 cyc/batch); logit
    rounding error from fp16 pf dominates anyway and stays ~1.4e-2.
  - origin_fmap streamed as fp8 E3M4 (4 mantissa bits, data ~N(0,1)):
    halves the of DMA bytes; mm2 takes the fp8 moving operand at fp16
    rate (mixed-dtype matmul), PSUM accumulates fp32.
  - TP=112 (T=100 padded) instead of 128 trims transpose/mm3 streams.
  - mm3 batched over groups of 4 batches (one [C,4*TP] PSUM tile) to
    amortize the wgT weight load.
  - software-pipelined schedule: step i emits [mm1(i) with attn-T(i-1)
    transposes interleaved] then [mm2(i-PIPE) with ctx-T(i-PIPE-1)
    interleaved + mm3]; the DMA queue is ordered pf0..pf2, (pf3,of0),
    (pf4,of1).. so arrivals are just-in-time for both phases.
  - per-batch PE floor reached: 16 stream MMs + 12 interleaved
    transposes + mm3/4 = ~5.3us/batch incl. instruction overheads.

Layout choices (host side prep is free):
  - position_fmap shipped pre-transposed per batch: pfT [B, 128, HT, L]
  - pos_emb shipped transposed fp16, T zero-padded to TP: peT [1, H, TP]
  - origin_fmap shipped l-tiled fp8e3: of [B, 128, LT, H]
  - W_gen shipped transposed: wgT [H, C]
  - output produced as [B, C, TP] fp16, transposed back on host.
"""

import numpy as np
import ml_dtypes

import concourse.mybir as mybir
import concourse.tile as tile
from concourse import bacc
from concourse.bass_utils import run_bass_kernel_spmd
from concourse.masks import make_identity

B, L, H, T, C = 64, 1024, 512, 100, 97
TP = 112
NCORES = 8
BPC = B // NCORES  # batches per core

HT = H // 128  # 4 h-tiles
LT = L // 128  # 8 l-tiles

F32 = mybir.dt.float32
AF = mybir.ActivationFunctionType
AX = mybir.AxisListType
OP = mybir.AluOpType

MM_DT = mybir.dt.float16
NP_DT = np.float16
OF_DT = mybir.dt.float8e3
OF_NP = ml_dtypes.float8_e3m4
PE_TERMS = 1
PIPE = 3  # back(b-PIPE) interleaves with front(b)
MM3_GRP = 4


def build_nc(mm_dt=MM_DT, of_dt=OF_DT, pe_terms=PE_TERMS, repeats=1, pipe=PIPE, hw_loop=None, only=None, dma_mode="single", unroll=8, order_swap=False, deep=False):
    """hw_loop=N wraps the body in a For_i hardware loop (timing builds):
    outT becomes an Internal scratch and a tiny token is the only external
    output, so per-call wire traffic is negligible and device time
    dominates."""
    timing = hw_loop is not None
    nc = bacc.Bacc(None, target_bir_lowering=False, debug=False)

    pfT = nc.dram_tensor("pfT", [BPC, 128, HT, L], mm_dt, kind="ExternalInput").ap()
    of = nc.dram_tensor("of", [BPC, 128, LT, H], of_dt, kind="ExternalInput").ap()
    peT = nc.dram_tensor("peT", [pe_terms, H, TP], mm_dt, kind="ExternalInput").ap()
    wgT = nc.dram_tensor("wgT", [H, C], mm_dt, kind="ExternalInput").ap()
    bg = nc.dram_tensor("bg", [C, 1], F32, kind="ExternalInput").ap()
    # outT is c-major [C, BPC, TP] so each 4-batch store writes per-partition
    # contiguous 896B runs instead of 224B scattered ones
    if timing:
        outT = nc.dram_tensor("outT", [C, BPC, TP], mm_dt, kind="Internal").ap()
        tok = nc.dram_tensor("tok", [C, 1], F32, kind="ExternalOutput").ap()
    else:
        outT = nc.dram_tensor("outT", [C, BPC, TP], mm_dt, kind="ExternalOutput").ap()

    with tile.TileContext(nc) as tc:
        with (
            tc.tile_pool(name="consts", bufs=1) as consts,
            tc.tile_pool(name="pf", bufs=BPC) as pfpool,
            tc.tile_pool(name="ofp", bufs=BPC) as ofpool,
            tc.tile_pool(name="mid", bufs=pipe + 3 if deep else pipe + 2) as mid,
            tc.tile_pool(name="work", bufs=4 if deep else 3) as work,
            tc.tile_pool(name="cts", bufs=2) as ctspool,
            tc.tile_pool(name="ps_scores", bufs=3, space="PSUM") as ps_scores,
            tc.tile_pool(name="ps_tp", bufs=2, space="PSUM") as ps_tp,
            tc.tile_pool(name="ps_ctx", bufs=2, space="PSUM") as ps_ctx,
            tc.tile_pool(name="ps_out", bufs=1, space="PSUM") as ps_out,
        ):
            # ---- constants ----
            peT_sb = consts.tile([128, pe_terms, HT, TP], mm_dt)
            peTr = peT.rearrange("e (ht p) t -> p e ht t", p=128)
            for e in range(pe_terms):
                nc.sync.dma_start(peT_sb[:, e], peTr[:, e])
            wgT_sb = consts.tile([128, HT, C], mm_dt)
            bg_sb = consts.tile([C, 1], F32)
            ident = consts.tile([128, 128], mm_dt)
            make_identity(nc, ident)

            def load_pf(b):
                pf_sb = pfpool.tile([128, HT, L], mm_dt, tag="pf")
                if dma_mode == "split4":
                    # per-ht DMAs: per-partition runs are contiguous 2KB
                    for ht in range(HT):
                        nc.sync.dma_start(pf_sb[:, ht, :], pfT[b, :, ht, :])
                elif dma_mode in ("single", "2q_big"):
                    # one 1MB dma_start; per-partition run = 8KB contiguous
                    nc.sync.dma_start(pf_sb, pfT[b])
                elif dma_mode == "act_single":
                    nc.scalar.dma_start(pf_sb, pfT[b])
                elif dma_mode == "halves":
                    nc.sync.dma_start(pf_sb[:, :2, :], pfT[b, :, :2, :])
                    nc.sync.dma_start(pf_sb[:, 2:, :], pfT[b, :, 2:, :])
                elif dma_mode == "2q":
                    nc.sync.dma_start(pf_sb[:, :2, :], pfT[b, :, :2, :])
                    nc.scalar.dma_start(pf_sb[:, 2:, :], pfT[b, :, 2:, :])
                elif dma_mode == "3q":
                    nc.sync.dma_start(pf_sb[:, :2, :], pfT[b, :, :2, :])
                    nc.scalar.dma_start(pf_sb[:, 2, :], pfT[b, :, 2, :])
                    nc.gpsimd.dma_start(pf_sb[:, 3, :], pfT[b, :, 3, :])
                return pf_sb

            def load_of(b):
                of_sb = ofpool.tile([128, LT, H], of_dt, tag="of")
                if dma_mode == "split4":
                    for i in range(4):
                        nc.sync.dma_start(
                            of_sb[:, 2 * i : 2 * (i + 1), :], of[b, :, 2 * i : 2 * (i + 1), :]
                        )
                elif dma_mode == "single":
                    nc.sync.dma_start(of_sb, of[b])
                elif dma_mode == "2q_big":
                    nc.scalar.dma_start(of_sb, of[b])
                elif dma_mode == "act_single":
                    nc.scalar.dma_start(of_sb, of[b])
                elif dma_mode == "halves":
                    nc.sync.dma_start(of_sb, of[b])
                elif dma_mode == "2q":
                    nc.sync.dma_start(of_sb[:, :4, :], of[b, :, :4, :])
                    nc.scalar.dma_start(of_sb[:, 4:, :], of[b, :, 4:, :])
                elif dma_mode == "3q":
                    nc.scalar.dma_start(of_sb[:, :4, :], of[b, :, :4, :])
                    nc.gpsimd.dma_start(of_sb[:, 4:, :], of[b, :, 4:, :])
                return of_sb

            def front(i, pf_sb, prevT):
                """mm1(i) with attn-T(i-1) transposes interleaved between the
                matmul streams (each transpose's LDWEIGHTS hides under the
                neighboring N=512 stream: ~366ns/pair vs 440 separate), then
                the softmax chain for i on DVE/ACT. Returns (p, rinv) of i;
                stores pT(i-1) into state."""
                tp_ps = None
                if prevT is not None:
                    p_prev, rinv_prev = prevT
                    tp_ps = ps_tp.tile([128, LT, TP], mm_dt, tag="tp", name="tp")
                    pT_sb = mid.tile([128, LT, TP], mm_dt, tag="pT", name="pT")
                k = 0
                if pf_sb is not None:
                    sc_ps = [
                        ps_scores.tile([TP, 512], F32, tag="scores", name=f"sc{lh}")
                        for lh in range(L // 512)
                    ]
                    for lh in range(L // 512):
                        for ht in range(HT):
                            nc.tensor.matmul(
                                sc_ps[lh],
                                lhsT=peT_sb[:, 0, ht, :],
                                rhs=pf_sb[:, ht, lh * 512 : (lh + 1) * 512],
                                start=(ht == 0),
                                stop=(ht == HT - 1),
                            )
                            if tp_ps is not None and k < LT:
                                nc.tensor.transpose(
                                    tp_ps[:, k, :],
                                    p_prev[:, k * 128 : (k + 1) * 128],
                                    ident[:TP, :TP],
                                )
                                k += 1
                if tp_ps is not None:
                    while k < LT:
                        nc.tensor.transpose(
                            tp_ps[:, k, :], p_prev[:, k * 128 : (k + 1) * 128], ident[:TP, :TP]
                        )
                        k += 1
                    half = LT // 2
                    nc.vector.tensor_copy(pT_sb[:, :half, :], tp_ps[:, :half, :])
                    nc.scalar.copy(pT_sb[:, half:, :], tp_ps[:, half:, :])
                    state[i - 1] = (pT_sb, rinv_prev)

                if pf_sb is None:
                    return None
                m2 = work.tile([TP, 2], F32, tag="m2")
                for lh in range(L // 512):
                    nc.vector.tensor_reduce(m2[:, lh : lh + 1], sc_ps[lh], axis=AX.X, op=OP.max)
                negm = work.tile([TP, 1], F32, tag="negm")
                nc.vector.tensor_reduce(negm, m2, axis=AX.X, op=OP.max, negate=True)
                p_sb = work.tile([TP, L], mm_dt, tag="p")
                s2 = work.tile([TP, 2], F32, tag="s2")
                for lh in range(L // 512):
                    nc.scalar.activation(
                        p_sb[:, lh * 512 : (lh + 1) * 512],
                        sc_ps[lh],
                        AF.Exp,
                        bias=negm,
                        scale=1.0,
                        accum_out=s2[:, lh : lh + 1],
                    )
                ssum = work.tile([TP, 1], F32, tag="ssum")
                nc.vector.tensor_reduce(ssum, s2, axis=AX.X, op=OP.add)
                rinv = mid.tile([TP, 1], F32, tag="rinv")
                nc.vector.reciprocal(rinv, ssum)
                return p_sb, rinv

            def back_mm2(bb, of_sb, pT_sb, rinv):
                """mm2(bb) with ctx-T(bb-1) transposes interleaved; then the
                rinv-scaled PSUM copy-out for bb."""
                prev_ctx = ctxs.pop(bb - 1, None)
                tp_ps = None
                if prev_ctx is not None:
                    tp_ps = ps_tp.tile([128, LT, TP], mm_dt, tag="tp", name="tpc")
                k = 0
                ctx_ps = ps_ctx.tile([TP, H], F32, tag="ctx")
                for lt in range(LT):
                    nc.tensor.matmul(
                        ctx_ps,
                        lhsT=pT_sb[:, lt, :],
                        rhs=of_sb[:, lt, :],
                        start=(lt == 0),
                        stop=(lt == LT - 1),
                    )
                    if tp_ps is not None and k < HT:
                        nc.tensor.transpose(
                            tp_ps[:, k, :],
                            prev_ctx[:, k * 128 : (k + 1) * 128],
                            ident[:TP, :TP],
                        )
                        k += 1
                if tp_ps is not None:
                    g = (bb - 1) // MM3_GRP
                    nc.scalar.copy(cT4[g][:, :, (bb - 1) % MM3_GRP, :], tp_ps[:, :HT, :])
                    if (bb - 1) % MM3_GRP == MM3_GRP - 1:
                        back_mm3(g * MM3_GRP, MM3_GRP, cT4[g])
                ctx_sb = work.tile([TP, H], mm_dt, tag="ctx_sb")
                nc.vector.tensor_scalar_mul(ctx_sb, ctx_ps[:], rinv)
                return ctx_sb

            def final_tp(bb):
                """drain: ctx transpose + copy + mm3 for the last group."""
                prev_ctx = ctxs.pop(bb)
                tp_ps = ps_tp.tile([128, LT, TP], mm_dt, tag="tp", name="tpc")
                for k in range(HT):
                    nc.tensor.transpose(
                        tp_ps[:, k, :], prev_ctx[:, k * 128 : (k + 1) * 128], ident[:TP, :TP]
                    )
                g = bb // MM3_GRP
                nc.scalar.copy(cT4[g][:, :, bb % MM3_GRP, :], tp_ps[:, :HT, :])
                back_mm3(g * MM3_GRP, BPC - g * MM3_GRP, cT4[g])

            def back_mm3(b0, nb, cT4_sb):
                """mm3 + bias + store for batches b0..b0+nb-1."""
                o_ps = ps_out.tile([C, MM3_GRP * TP], F32, tag="o")
                for ht in range(HT):
                    nc.tensor.matmul(
                        o_ps[:, : nb * TP],
                        lhsT=wgT_sb[:, ht, :],
                        rhs=cT4_sb[:, ht, :nb, :],
                        start=(ht == 0),
                        stop=(ht == HT - 1),
                    )
                out_sb = work.tile([C, MM3_GRP, TP], mm_dt, tag="out_sb")
                nc.vector.tensor_scalar_add(
                    out_sb[:, :nb, :],
                    o_ps[:, : nb * TP].rearrange("c (b t) -> c b t", b=nb),
                    bg_sb,
                )
                nc.gpsimd.dma_start(outT[:, b0 : b0 + nb, :], out_sb[:, :nb, :])

            state = {}
            ctxs = {}
            cT4 = {}

            nodma_tiles = None
            if only == "nodma":
                pf0_sb = consts.tile([128, HT, L], mm_dt, name="pf0c")
                of0_sb = consts.tile([128, LT, H], of_dt, name="of0c")
                nc.sync.dma_start(pf0_sb, pfT[0])
                nc.sync.dma_start(of0_sb, of[0])
                nodma_tiles = (pf0_sb, of0_sb)

            def body(load_consts, warm=False):
                # (HAM warmup via throwaway matmuls was tried here and
                # measured 2.4us/rep SLOWER - do not re-add.)
                if only == "empty":
                    nc.vector.tensor_copy(bg_sb, bg_sb)
                    return
                if only == "pe":
                    # dense independent matmuls: 64 x 512 rows = 32768 PE
                    # cycles -> 13.6us warm / 27.3us cold
                    pf_sb = pfpool.tile([128, HT, L], mm_dt, tag="pf", name="pf")
                    nc.sync.dma_start(pf_sb, pfT[0])
                    for i in range(64):
                        sc = ps_scores.tile([TP, 512], F32, tag="scores", name="sc")
                        nc.tensor.matmul(
                            sc,
                            lhsT=peT_sb[:, 0, i % HT, :],
                            rhs=pf_sb[:, i % HT, (i % 2) * 512 : (i % 2 + 1) * 512],
                            start=True,
                            stop=True,
                        )
                    return
                if only == "pe_grp":
                    # 16 groups of 4 accumulating MMs (64 MMs, N=512)
                    pf_sb = pfpool.tile([128, HT, L], mm_dt, tag="pf", name="pf")
                    nc.sync.dma_start(pf_sb, pfT[0])
                    for g in range(16):
                        sc = ps_scores.tile([TP, 512], F32, tag="scores", name="sc")
                        for j in range(4):
                            nc.tensor.matmul(
                                sc,
                                lhsT=peT_sb[:, 0, j, :],
                                rhs=pf_sb[:, j, (g % 2) * 512 : (g % 2 + 1) * 512],
                                start=(j == 0),
                                stop=(j == 3),
                            )
                    return
                if only == "pe_tp":
                    # 64 transposes of [TP,128] -> overhead probe
                    p_sb = work.tile([TP, L], mm_dt, tag="p", name="p")
                    nc.sync.dma_start(p_sb, pfT[0, :TP, 0, :])
                    for i in range(64):
                        tp_ps = ps_tp.tile([128, TP], mm_dt, tag="tp", name="tp")
                        nc.tensor.transpose(
                            tp_ps, p_sb[:, (i % 8) * 128 : (i % 8 + 1) * 128], ident[:TP, :TP]
                        )
                    return
                if only == "pe_n1024":
                    # 32 MMs with N=1024 out (2 PSUM banks) - legality probe
                    pf_sb = pfpool.tile([128, HT, L], mm_dt, tag="pf", name="pf")
                    nc.sync.dma_start(pf_sb, pfT[0])
                    for i in range(32):
                        sc = ps_scores.tile([TP, 1024], F32, tag="sc1k", name="sc")
                        nc.tensor.matmul(
                            sc,
                            lhsT=peT_sb[:, 0, i % HT, :],
                            rhs=pf_sb[:, i % HT, :],
                            start=True,
                            stop=True,
                        )
                    return
                if only == "pe_acc":
                    # 64 MMs N=512 accumulating into ONE psum tile: no
                    # per-MM WAR semaphores, one group
                    pf_sb = pfpool.tile([128, HT, L], mm_dt, tag="pf", name="pf")
                    nc.sync.dma_start(pf_sb, pfT[0])
                    sc = ps_scores.tile([TP, 512], F32, tag="scores", name="sc")
                    for i in range(64):
                        nc.tensor.matmul(
                            sc,
                            lhsT=peT_sb[:, 0, i % HT, :],
                            rhs=pf_sb[:, i % HT, (i % 2) * 512 : (i % 2 + 1) * 512],
                            start=(i == 0),
                            stop=(i == 63),
                        )
                    nc.vector.tensor_reduce(
                        work.tile([TP, 1], F32, tag="m2", name="m2"), sc, axis=AX.X, op=OP.max
                    )
                    return
                if only == "pe_same":
                    # 64 MMs N=512, all with the SAME stationary operand
                    pf_sb = pfpool.tile([128, HT, L], mm_dt, tag="pf", name="pf")
                    nc.sync.dma_start(pf_sb, pfT[0])
                    for i in range(64):
                        sc = ps_scores.tile([TP, 512], F32, tag="scores", name="sc")
                        nc.tensor.matmul(
                            sc,
                            lhsT=peT_sb[:, 0, 0, :],
                            rhs=pf_sb[:, i % HT, (i % 2) * 512 : (i % 2 + 1) * 512],
                            start=True,
                            stop=True,
                        )
                    return
                if only == "pe_mix":
                    # 32 MMs N=512 alternating with 32 transposes
                    pf_sb = pfpool.tile([128, HT, L], mm_dt, tag="pf", name="pf")
                    nc.sync.dma_start(pf_sb, pfT[0])
                    p_sb = work.tile([TP, L], mm_dt, tag="p", name="p")
                    nc.sync.dma_start(p_sb, pfT[0, :TP, 0, :])
                    for i in range(32):
                        sc = ps_scores.tile([TP, 512], F32, tag="scores", name="sc")
                        nc.tensor.matmul(
                            sc,
                            lhsT=peT_sb[:, 0, i % HT, :],
                            rhs=pf_sb[:, i % HT, (i % 2) * 512 : (i % 2 + 1) * 512],
                            start=True,
                            stop=True,
                        )
                        tp_ps = ps_tp.tile([128, TP], mm_dt, tag="tp", name="tp")
                        nc.tensor.transpose(
                            tp_ps, p_sb[:, (i % 8) * 128 : (i % 8 + 1) * 128], ident[:TP, :TP]
                        )
                    return
                # ---- DMA queue order: JIT arrivals ----
                pf_tiles = {}
                of_tiles = {}
                if only == "nodma":
                    for b in range(BPC):
                        pf_tiles[b] = nodma_tiles[0]
                        of_tiles[b] = nodma_tiles[1]
                else:
                    for b in range(min(pipe, BPC)):
                        pf_tiles[b] = load_pf(b)
                    if load_consts:
                        nc.sync.dma_start(wgT_sb, wgT.rearrange("(ht p) c -> p ht c", p=128))
                        nc.sync.dma_start(bg_sb, bg)
                    for b in range(pipe, BPC):
                        pf_tiles[b] = load_pf(b)
                        of_tiles[b - pipe] = load_of(b - pipe)
                    for b in range(max(BPC - pipe, 0), BPC):
                        of_tiles[b] = load_of(b)
                if only == "dma":
                    return

                # ---- compute: PE order per step i is
                #   [mm1(i) ⊗ attn-T(i-1)] | [mm2(i-pipe) ⊗ ctx-T(i-pipe-1), mm3]
                # (⊗ = transposes interleaved between matmul streams so their
                # LDWEIGHTS hide; the softmax chain of batch i hides under
                # the back-half matmuls). ----
                state.clear()
                ctxs.clear()
                cT4.clear()
                prev = None
                for i in range(BPC + pipe):
                    if i < BPC:
                        prev = front(i, pf_tiles.pop(i), prev)
                    elif i == BPC:
                        front(i, None, prev)
                        prev = None
                    if i >= pipe:
                        bb = i - pipe
                        if bb % MM3_GRP == 0:
                            cT4[bb // MM3_GRP] = ctspool.tile(
                                [128, HT, MM3_GRP, TP], mm_dt, tag="cT4", name="cT4"
                            )
                        ctxs[bb] = back_mm2(bb, of_tiles.pop(bb), *state.pop(bb))
                final_tp(BPC - 1)

            if timing:
                # unroll several reps per For_i iteration: the loop's
                # all-engine barrier idles PE long enough to re-engage the
                # HAM clock throttle (1.2 GHz), so amortize it to ~1% and
                # measure warm steady-state throughput like an unrolled NEFF
                assert hw_loop % unroll == 0
                nc.sync.dma_start(wgT_sb, wgT.rearrange("(ht p) c -> p ht c", p=128))
                nc.sync.dma_start(bg_sb, bg)
                with tc.For_i(0, hw_loop // unroll, 1):
                    for _u in range(unroll):
                        body(load_consts=False)
                nc.gpsimd.dma_start(tok, bg_sb)
            else:
                for _rep in range(repeats):
                    body(load_consts=(_rep == 0))

    nc.compile()
    return nc


_NC = None


def _get_nc():
    global _NC
    if _NC is None:
        _NC = build_nc()
    return _NC


def make_in_maps(position_fmap, origin_fmap, pos_emb, W_gen, b_gen, np_dt=NP_DT, pe_terms=PE_TERMS, of_np=OF_NP):
    """Host-side sharding + layout prep. Returns list of per-core input dicts."""
    pf = np.asarray(position_fmap, dtype=np.float32)
    of = np.asarray(origin_fmap, dtype=np.float32)
    pe = np.asarray(pos_emb, dtype=np.float32)
    wg = np.asarray(W_gen, dtype=np.float32)
    bgv = np.asarray(b_gen, dtype=np.float32)

    # [B, L, H] -> [B, H, L] -> [B, 128, HT, L]  (partition-major, h = ht*128 + p)
    pfT = np.ascontiguousarray(
        pf.transpose(0, 2, 1).reshape(B, HT, 128, L).transpose(0, 2, 1, 3)
    ).astype(np_dt)
    # [B, L, H] -> [B, 128, LT, H]  (partition-major tiling, l = lt*128 + p)
    of_c = np.ascontiguousarray(
        of.reshape(B, LT, 128, H).transpose(0, 2, 1, 3)
    ).astype(of_np)

    peT_f32 = np.zeros((H, TP), dtype=np.float32)
    peT_f32[:, :T] = pe.T
    terms = []
    resid = peT_f32
    for _ in range(pe_terms):
        t = resid.astype(np_dt)
        terms.append(t)
        resid = resid - t.astype(np.float32)
    peT = np.ascontiguousarray(np.stack(terms, axis=0))  # [pe_terms, H, TP]

    wgT = np.ascontiguousarray(wg.T).astype(np_dt)
    bg2 = np.ascontiguousarray(bgv.reshape(C, 1)).astype(np.float32)

    in_maps = []
    for i in range(NCORES):
        sl = slice(i * BPC, (i + 1) * BPC)
        in_maps.append(
            {
                "pfT": pfT[sl],
                "of": of_c[sl],
                "peT": peT,
                "wgT": wgT,
                "bg": bg2,
            }
        )
    return in_maps


def kernel(position_fmap, origin_fmap, pos_emb, W_gen, b_gen):
    nc = _get_nc()
    in_maps = make_in_maps(position_fmap, origin_fmap, pos_emb, W_gen, b_gen)
    res = run_bass_kernel_spmd(nc, in_maps, core_ids=list(range(NCORES)))
    outs = [r["outT"] for r in res.results]  # each [C, BPC, TP]
    out = np.concatenate(outs, axis=1)  # [C, B, TP]
    return np.ascontiguousarray(out.transpose(1, 2, 0)[:, :T, :]).astype(np.float32)
